# revision 2
# baseline (speedup 1.0000x reference)
"""Trainium2 Bass kernel for nn_AutoregressiveMixerBlock.

Reference computation (per batch b):
  y  = LN_H(x)                                    # layer norm over H
  t  = revcumsum_N(y)                             # t[j] = sum_{i>=j} y[i]
  h  = gelu(t^T @ tok_w1 + tok_b1)                # [H, TM]
  y2 = (h @ tok_w2 + tok_b2)^T                    # [N, H]
  y3 = LN_H(y2)
  out = gelu(y3 @ ch_w1 + ch_b1) @ ch_w2 + ch_b2  # [N, H]

Key algebraic folds (exact in real arithmetic, applied on host):
  * revcumsum+matmul:  sum_j t[j,h] w1[j,m] = sum_i y[i,h] W1c[i,m]
    with W1c = cumsum(tok_w1, axis=0) -> no on-device cumsum at all.
  * LN1 gain/bias move past the token matmul:
    out1[h,m] = g[h] * (yn^T @ W1c)[h,m] + (b[h]*colsum1[m] + tok_b1[m])
  * tok_b2 and the LN2 mean both vanish by centering h^T by its
    per-row (over H) mean before the second token matmul.
  * LN2 gain/bias fold into ch_w1 / ch_b1.

Sharding: data-parallel over B across 8 cores (2 batches per core),
weights replicated.
"""

import numpy as np

B, N, H = 16, 8192, 128
TM, CM = 256, 512
EPS = 1e-5
NCORES = 8
BL = B // NCORES          # batches per core
P = 128                   # partitions
NC_TOK = N // P           # 64 token chunks of 128
NJ = N // 512             # 16 column chunks of 512
KTM = TM // P             # 2 k-chunks for the second token matmul
NCI = CM // P             # 4 chunks of the channel hidden dim

_cached = {}


def _build(nontrivial_bias1, nontrivial_cb2):
    import concourse.bass as bass
    import concourse.mybir as mybir
    import concourse.tile as tile
    from concourse import bacc
    from concourse.masks import make_identity
    import bass_rust

    F32 = mybir.dt.float32
    F32R = mybir.dt.float32r
    BF16 = mybir.dt.bfloat16
    AF = mybir.ActivationFunctionType
    ALU = mybir.AluOpType
    AX = mybir.AxisListType

    nc = bacc.Bacc()

    # ---- DRAM tensors -------------------------------------------------
    x_d = nc.dram_tensor("x", [BL, N, H], F32, kind="ExternalInput")
    w1c_d = nc.dram_tensor("w1c", [N, TM], F32R, kind="ExternalInput")
    w2_d = nc.dram_tensor("w2", [TM, N], F32R, kind="ExternalInput")
    g1_d = nc.dram_tensor("g1", [P, 1], F32, kind="ExternalInput")
    bias1_d = nc.dram_tensor("bias1", [P, TM], F32, kind="ExternalInput")
    cw1_d = nc.dram_tensor("cw1", [H, CM], BF16, kind="ExternalInput")
    cb1_d = nc.dram_tensor("cb1", [P, NCI], F32, kind="ExternalInput")
    cw2_d = nc.dram_tensor("cw2", [CM, H], BF16, kind="ExternalInput")
    cb2_d = nc.dram_tensor("cb2", [P, 1], F32, kind="ExternalInput")
    ones_d = nc.dram_tensor("ones", [P, P], F32R, kind="ExternalInput")
    out_d = nc.dram_tensor("out", [BL, H, N], F32, kind="ExternalOutput")

    # DRAM views
    x_v = [x_d[b].rearrange("(c p) h -> p c h", p=P) for b in range(BL)]
    w1c_v = w1c_d[:].rearrange("(c p) m -> p c m", p=P)
    w2_v = w2_d[:].rearrange("(k p) (j n) -> p k j n", p=P, n=512)
    cw2_v = cw2_d[:].rearrange("(ci p) h -> p ci h", p=P)
    out_v = [out_d[b] for b in range(BL)]

    act_phases = [[], [], [], []]  # ACT table-set phase buckets

    with tile.TileContext(nc) as tc:
        import contextlib
        with contextlib.ExitStack() as ctx:
            const = ctx.enter_context(tc.tile_pool(name="const", bufs=1))
            xall = ctx.enter_context(tc.tile_pool(name="xall", bufs=BL))
            stats = ctx.enter_context(tc.tile_pool(name="stats", bufs=2 * BL))
            small = ctx.enter_context(tc.tile_pool(name="small", bufs=4))
            sqp = ctx.enter_context(tc.tile_pool(name="sqp", bufs=1))
            w1cs = ctx.enter_context(tc.tile_pool(name="w1cs", bufs=4))
            w2s = ctx.enter_context(tc.tile_pool(name="w2s", bufs=6))
            sq2p = ctx.enter_context(tc.tile_pool(name="sq2p", bufs=3))
            rstdp = ctx.enter_context(tc.tile_pool(name="rstdp", bufs=3))
            g2p = ctx.enter_context(tc.tile_pool(name="g2p", bufs=2))
            outp = ctx.enter_context(tc.tile_pool(name="outp", bufs=3))

            # ---- constants -------------------------------------------
            g1_sb = const.tile([P, 1], F32)
            nc.sync.dma_start(g1_sb, g1_d[:])
            cw1_sb = const.tile([H, CM], BF16)
            nc.sync.dma_start(cw1_sb, cw1_d[:])
            cb1_sb = const.tile([P, NCI], F32)
            nc.sync.dma_start(cb1_sb, cb1_d[:])
            cw2_sb = const.tile([P, NCI, H], BF16)
            nc.sync.dma_start(cw2_sb, cw2_v)
            ones_sb = const.tile([P, P], F32R)
            nc.sync.dma_start(ones_sb, ones_d[:])
            ident = const.tile([P, P], F32)
            make_identity(nc, ident)
            if nontrivial_bias1:
                bias1_sb = const.tile([P, TM], F32)
                nc.sync.dma_start(bias1_sb, bias1_d[:])
            if nontrivial_cb2:
                cb2_sb = const.tile([P, 1], F32)
                nc.sync.dma_start(cb2_sb, cb2_d[:])
                cb2_t = small.tile([P, 1], F32, tag="cb2t")
                nc.vector.tensor_copy(cb2_t, cb2_sb)
            # pre-touch the per-partition scalar so later scalar-pointer
            # ops don't need a DMA wait of their own
            g1_t = small.tile([P, 1], F32)
            nc.vector.tensor_copy(g1_t, g1_sb)
            eps_t = const.tile([P, 1], F32)
            nc.vector.memset(eps_t, EPS)

            # ---- phase 1: LN1 stats + normalize + token matmul 1 -----
            x_sb = []
            rstd1 = []
            mu1 = []
            for b in range(BL):
                xt = xall.tile([P, NC_TOK, H], F32, tag="xall", name=f"xall{b}")
                nc.sync.dma_start(xt, x_v[b])
                x_sb.append(xt)

                sums = stats.tile([P, NC_TOK], F32, tag="st_sum")
                nc.vector.tensor_reduce(
                    out=sums, in_=xt, axis=AX.X, op=ALU.add)
                sq = sqp.tile([P, NC_TOK, H], BF16, tag="sq")
                i_sq = nc.scalar.activation(sq, xt, AF.Square)
                act_phases[0].append(i_sq)
                sumsq = stats.tile([P, NC_TOK], F32, tag="st_sumsq")
                nc.vector.tensor_reduce(
                    out=sumsq, in_=sq, axis=AX.X, op=ALU.add)

                mu = stats.tile([P, NC_TOK], F32, tag="st_mu")
                nc.vector.tensor_scalar_mul(mu, sums, 1.0 / H)
                ex2 = stats.tile([P, NC_TOK], F32, tag="st_ex2")
                nc.vector.tensor_scalar_mul(ex2, sumsq, 1.0 / H)
                musq = stats.tile([P, NC_TOK], F32, tag="st_musq")
                nc.vector.tensor_tensor(musq, mu, mu, ALU.mult)
                var = stats.tile([P, NC_TOK], F32, tag="st_var")
                nc.vector.tensor_tensor(var, ex2, musq, ALU.subtract)
                nc.vector.tensor_scalar(
                    out=var, in0=var, scalar1=EPS, scalar2=None, op0=ALU.add)
                std = stats.tile([P, NC_TOK], F32, tag="st_std")
                i_r = nc.scalar.activation(std, var, AF.Sqrt)
                act_phases[0].append(i_r)
                rst = stats.tile([P, NC_TOK], F32, tag="st_rstd")
                nc.vector.reciprocal_approx_fast(rst, std)
                rstd1.append(rst)
                mu1.append(mu)

            with (
                tc.tile_pool(name="ps1", bufs=BL, space="PSUM") as ps1,
                tc.tile_pool(name="pst", bufs=2, space="PSUM") as pst,
            ):
                psum1 = [ps1.tile([P, TM], F32, tag="ps1", name=f"ps1_{b}")
                         for b in range(BL)]
                for c in range(NC_TOK):
                    w1t = w1cs.tile([P, TM], F32R, tag="w1c")
                    nc.sync.dma_start(w1t, w1c_v[:, c, :])
                    for b in range(BL):
                        xn = small.tile([P, P], F32R, tag="xn")
                        nc.vector.tensor_scalar(
                            out=xn,
                            in0=x_sb[b][:, c, :],
                            scalar1=mu1[b][:, c:c + 1],
                            scalar2=rstd1[b][:, c:c + 1],
                            op0=ALU.subtract,
                            op1=ALU.mult,
                        )
                        nc.tensor.matmul(
                            psum1[b],
                            xn,
                            w1t,
                            start=(c == 0),
                            stop=(c == NC_TOK - 1),
                        )

                # ---- phase 2: token gelu, transpose, center --------------
                h1c = []  # per batch: list of KTM [P, P] f32r tiles
                for b in range(BL):
                    h1 = small.tile([P, TM], F32, tag="h1")
                    if nontrivial_bias1:
                        nc.vector.tensor_scalar_mul(h1, psum1[b], g1_t)
                        nc.vector.tensor_add(h1, h1, bias1_sb)
                        i_g = nc.scalar.activation(h1, h1, AF.Gelu)
                    else:
                        i_g = nc.scalar.activation(h1, psum1[b], AF.Gelu,
                                                   scale=g1_t)
                    act_phases[1].append(i_g)

                    chunks = []
                    for k in range(KTM):
                        ps_t = pst.tile([P, P], F32, tag="pst")
                        nc.tensor.transpose(ps_t, h1[:, k * P:(k + 1) * P], ident)
                        h1T = small.tile([P, P], F32, tag="h1T")
                        nc.vector.tensor_copy(h1T, ps_t)
                        hsum = small.tile([P, 1], F32, tag="hsum")
                        nc.vector.tensor_reduce(
                            out=hsum, in_=h1T, axis=AX.X, op=ALU.add)
                        hmean = small.tile([P, 1], F32, tag="hmean")
                        nc.vector.tensor_scalar_mul(hmean, hsum, 1.0 / H)
                        hc = small.tile([P, P], F32R, tag="h1c")
                        nc.vector.tensor_scalar(
                            out=hc, in0=h1T, scalar1=hmean, scalar2=None,
                            op0=ALU.subtract)
                        chunks.append(hc)
                    h1c.append(chunks)

            # ---- phase 3a: token matmul 2 + LN2 stats ----------------
            with (
                tc.tile_pool(name="ps2", bufs=3, space="PSUM") as ps2,
                tc.tile_pool(name="psv", bufs=2, space="PSUM") as psv,
            ):
                y2n = []
                for b in range(BL):
                    y2n.append(xall.tile([P, N], BF16, tag="xall", name=f"y2n{b}"))

                for j in range(NJ):
                    w2t = []
                    for k in range(KTM):
                        wt = w2s.tile([P, 512], F32R, tag="w2")
                        nc.sync.dma_start(wt, w2_v[:, k, j, :])
                        w2t.append(wt)
                    for b in range(BL):
                        p2 = ps2.tile([P, 512], F32, tag="ps2")
                        for k in range(KTM):
                            nc.tensor.matmul(
                                p2, h1c[b][k], w2t[k],
                                start=(k == 0), stop=(k == KTM - 1))
                        sq2 = sq2p.tile([P, 512], F32R, tag="sq2")
                        i_s = nc.scalar.activation(
                            sq2, p2, AF.Square, scale=float(1.0 / np.sqrt(H)))
                        act_phases[2].append(i_s)
                        vps = psv.tile([P, 512], F32, tag="psv")
                        nc.tensor.matmul(vps, ones_sb, sq2, start=True, stop=True)
                        std = rstdp.tile([P, 512], F32, tag="std")
                        i_r = nc.scalar.activation(std, vps, AF.Sqrt, bias=eps_t)
                        act_phases[2].append(i_r)
                        rstd = rstdp.tile([P, 512], F32, tag="rstd")
                        nc.vector.reciprocal_approx_fast(rstd, std)
                        nc.vector.tensor_tensor(
                            y2n[b][:, j * 512:(j + 1) * 512],
                            p2, rstd, ALU.mult)

            # ---- phase 3b: channel MLP -------------------------------
            with (
                tc.tile_pool(name="psr", bufs=1, space="PSUM") as psr,
                tc.tile_pool(name="pso", bufs=2, space="PSUM") as pso,
            ):
                for j in range(NJ):
                    for b in range(BL):
                        y2s = y2n[b][:, j * 512:(j + 1) * 512]
                        raw2 = psr.tile([P, NCI * 512], F32, tag="psr")
                        for ci in range(NCI):
                            nc.tensor.matmul(
                                raw2[:, ci * 512:(ci + 1) * 512],
                                cw1_sb[:, ci * P:(ci + 1) * P],
                                y2s, start=True, stop=True)
                        g2 = g2p.tile([P, NCI * 512], BF16, tag="g2")
                        if nontrivial_bias1:
                            # general path: per-ci bias
                            for ci in range(NCI):
                                i_g = nc.scalar.activation(
                                    g2[:, ci * 512:(ci + 1) * 512],
                                    raw2[:, ci * 512:(ci + 1) * 512],
                                    AF.Gelu, bias=cb1_sb[:, ci:ci + 1])
                                act_phases[3].append(i_g)
                        else:
                            i_g = nc.scalar.activation(g2, raw2, AF.Gelu)
                            act_phases[3].append(i_g)

                        po = pso.tile([P, 512], F32, tag="pso")
                        for ci in range(NCI):
                            nc.tensor.matmul(
                                po,
                                cw2_sb[:, ci, :],
                                g2[:, ci * 512:(ci + 1) * 512],
                                start=(ci == 0), stop=(ci == NCI - 1))
                        osb = outp.tile([P, 512], F32, tag="osb")
                        if nontrivial_cb2:
                            nc.vector.tensor_scalar(
                                out=osb, in0=po, scalar1=cb2_t, scalar2=None,
                                op0=ALU.add)
                        else:
                            nc.vector.tensor_copy(osb, po)
                        nc.sync.dma_start(
                            out_v[b][:, j * 512:(j + 1) * 512], osb)

            # ---- ACT table-set ordering edges ------------------------
            for ph in range(3):
                for f in act_phases[ph + 1]:
                    for t in act_phases[ph]:
                        bass_rust.add_dep_helper(
                            f.ins, t.ins, sync=False,
                            reason="act table set phase ordering")

    nc.compile()
    return nc


def _host_prep(inputs):
    x = np.ascontiguousarray(inputs["x"], dtype=np.float32)
    ln1_g = np.asarray(inputs["ln1_g"], np.float32)
    ln1_b = np.asarray(inputs["ln1_b"], np.float32)
    ln2_g = np.asarray(inputs["ln2_g"], np.float32)
    ln2_b = np.asarray(inputs["ln2_b"], np.float32)
    tok_w1 = np.asarray(inputs["tok_w1"], np.float32)
    tok_b1 = np.asarray(inputs["tok_b1"], np.float32)
    tok_w2 = np.asarray(inputs["tok_w2"], np.float32)
    ch_w1 = np.asarray(inputs["ch_w1"], np.float32)
    ch_b1 = np.asarray(inputs["ch_b1"], np.float32)
    ch_w2 = np.asarray(inputs["ch_w2"], np.float32)
    ch_b2 = np.asarray(inputs["ch_b2"], np.float32)

    import ml_dtypes
    w1c = np.cumsum(tok_w1, axis=0, dtype=np.float64).astype(np.float32)
    colsum1 = w1c.sum(axis=0, dtype=np.float64).astype(np.float32)
    bias1 = ln1_b[:, None] * colsum1[None, :] + tok_b1[None, :]
    cw1 = (ln2_g[:, None] * ch_w1).astype(np.float32)
    cb1 = (ch_b1 + ch_w1.T @ ln2_b).astype(np.float32)
    cw2 = ch_w2.astype(ml_dtypes.bfloat16)


    nontrivial_bias1 = bool(np.any(bias1 != 0.0) or np.any(cb1 != 0.0))
    nontrivial_cb2 = bool(np.any(ch_b2 != 0.0))

    shared = {
        "w1c": w1c,
        "w2": np.ascontiguousarray(tok_w2),
        "g1": ln1_g.reshape(P, 1).copy(),
        "bias1": np.ascontiguousarray(bias1, np.float32),
        "cw1": cw1.astype(ml_dtypes.bfloat16),
        "cb1": np.ascontiguousarray(cb1.reshape(NCI, P).T.copy()),
        "cw2": np.ascontiguousarray(cw2),
        "cb2": ch_b2.reshape(P, 1).astype(np.float32).copy(),
        "ones": np.ones((P, P), np.float32),
    }
    return x, shared, nontrivial_bias1, nontrivial_cb2


def _assemble(r):
    return r["out"].transpose(0, 2, 1)


def kernel(**inputs) -> np.ndarray:
    from concourse.bass_utils import run_bass_kernel_spmd

    x, shared, nb1, nb2 = _host_prep(inputs)

    key = (nb1, nb2)
    if key not in _cached:
        _cached[key] = _build(nb1, nb2)
    nc = _cached[key]

    in_maps = []
    for c in range(NCORES):
        m = dict(shared)
        m["x"] = np.ascontiguousarray(x[c * BL:(c + 1) * BL])
        in_maps.append(m)

    res = run_bass_kernel_spmd(nc, in_maps, core_ids=list(range(NCORES)))
    out = np.concatenate([_assemble(r) for r in res.results], axis=0)
    return np.ascontiguousarray(out, dtype=np.float32)


if __name__ == "__main__":
    rng = np.random.default_rng(0)
    ins = {
        "x": rng.standard_normal((B, N, H)).astype(np.float32),
        "ln1_g": np.ones(H, np.float32),
        "ln1_b": np.zeros(H, np.float32),
        "ln2_g": np.ones(H, np.float32),
        "ln2_b": np.zeros(H, np.float32),
        "tok_w1": (rng.standard_normal((N, TM)) * 0.02).astype(np.float32),
        "tok_b1": np.zeros(TM, np.float32),
        "tok_w2": (rng.standard_normal((TM, N)) * 0.02).astype(np.float32),
        "tok_b2": np.zeros(N, np.float32),
        "ch_w1": (rng.standard_normal((H, CM)) * 0.02).astype(np.float32),
        "ch_b1": np.zeros(CM, np.float32),
        "ch_w2": (rng.standard_normal((CM, H)) * 0.02).astype(np.float32),
        "ch_b2": np.zeros(H, np.float32),
    }
    out = kernel(**ins)
    print("out", out.shape, out.dtype)



# revision 3
# speedup vs baseline: 1.0419x; 1.0419x over previous
"""Trainium2 Bass kernel for nn_AutoregressiveMixerBlock.

Reference computation (per batch b):
  y  = LN_H(x)                                    # layer norm over H
  t  = revcumsum_N(y)                             # t[j] = sum_{i>=j} y[i]
  h  = gelu(t^T @ tok_w1 + tok_b1)                # [H, TM]
  y2 = (h @ tok_w2 + tok_b2)^T                    # [N, H]
  y3 = LN_H(y2)
  out = gelu(y3 @ ch_w1 + ch_b1) @ ch_w2 + ch_b2  # [N, H]

Key algebraic folds (exact in real arithmetic, applied on host):
  * revcumsum+matmul:  sum_j t[j,h] w1[j,m] = sum_i y[i,h] W1c[i,m]
    with W1c = cumsum(tok_w1, axis=0) -> no on-device cumsum at all.
  * LN1 gain/bias move past the token matmul.
  * tok_b2 and the LN2 mean both vanish by centering h^T by its
    per-row (over H) mean before the second token matmul.
  * LN2 gain/bias fold into ch_w1 / ch_b1.

Perf notes vs the f32r baseline:
  * every matmul operand is bf16 (f32 HIGH-mode matmuls ran at ~2.5x
    the bf16 row rate and power-throttled the PE to 50%).
  * weights live SBUF-resident, DMA'd once in 16 chunks each with
    per-partition-contiguous host layouts (big packets, all queues).
  * x is pre-arranged on host to [P, NC, H] bf16 per batch.
  * LN1's square runs on DVE (bf16 2x/4x modes) instead of ACT.
  * output is written bf16 and converted on host.

Sharding: data-parallel over B across 8 cores (2 batches per core),
weights replicated.
"""

import numpy as np

B, N, H = 16, 8192, 128
TM, CM = 256, 512
EPS = 1e-5
NCORES = 8
BL = B // NCORES          # batches per core
P = 128                   # partitions
NC_TOK = N // P           # 64 token chunks of 128
NJ = N // 512             # 16 column chunks of 512
KTM = TM // P             # 2 k-chunks for the second token matmul
NCI = CM // P             # 4 chunks of the channel hidden dim

_cached = {}


def _build(nontrivial_bias1, nontrivial_cb2):
    import concourse.bass as bass
    import concourse.mybir as mybir
    import concourse.tile as tile
    from concourse import bacc
    from concourse.masks import make_identity
    import bass_rust

    F32 = mybir.dt.float32
    BF16 = mybir.dt.bfloat16
    AF = mybir.ActivationFunctionType
    ALU = mybir.AluOpType
    AX = mybir.AxisListType

    nc = bacc.Bacc()

    # ---- DRAM tensors -------------------------------------------------
    # x pre-arranged on host: x[b, p, c, h] = orig_x[b, c*128+p, h]
    x_d = nc.dram_tensor("x", [BL, P, NC_TOK, H], BF16, kind="ExternalInput")
    # w1c pre-arranged: w1c[p, c, m] = cumsum(tok_w1)[c*128+p, m]
    w1c_d = nc.dram_tensor("w1c", [P, NC_TOK, TM], BF16, kind="ExternalInput")
    # w2 pre-arranged: w2[p, k, j, n] = tok_w2[k*128+p, j*512+n]
    w2_d = nc.dram_tensor("w2", [P, KTM, NJ, 512], BF16, kind="ExternalInput")
    g1_d = nc.dram_tensor("g1", [P, 1], F32, kind="ExternalInput")
    bias1_d = nc.dram_tensor("bias1", [P, TM], F32, kind="ExternalInput")
    cw1_d = nc.dram_tensor("cw1", [H, CM], BF16, kind="ExternalInput")
    cb1_d = nc.dram_tensor("cb1", [P, NCI], F32, kind="ExternalInput")
    cw2_d = nc.dram_tensor("cw2", [P, NCI, H], BF16, kind="ExternalInput")
    cb2_d = nc.dram_tensor("cb2", [P, 1], F32, kind="ExternalInput")
    ones_d = nc.dram_tensor("ones", [P, P], BF16, kind="ExternalInput")
    out_d = nc.dram_tensor("out", [BL, H, N], BF16, kind="ExternalOutput")

    out_v = [out_d[b] for b in range(BL)]

    act_phases = [[], [], [], []]  # ACT table-set phase buckets

    with tile.TileContext(nc) as tc:
        import contextlib
        with contextlib.ExitStack() as ctx:
            const = ctx.enter_context(tc.tile_pool(name="const", bufs=1))
            xall = ctx.enter_context(tc.tile_pool(name="xall", bufs=BL))
            stats = ctx.enter_context(tc.tile_pool(name="stats", bufs=2 * BL))
            small = ctx.enter_context(tc.tile_pool(name="small", bufs=4))
            sqp = ctx.enter_context(tc.tile_pool(name="sqp", bufs=1))
            sq2p = ctx.enter_context(tc.tile_pool(name="sq2p", bufs=3))
            rstdp = ctx.enter_context(tc.tile_pool(name="rstdp", bufs=3))
            g2p = ctx.enter_context(tc.tile_pool(name="g2p", bufs=2))
            outp = ctx.enter_context(tc.tile_pool(name="outp", bufs=3))

            # ---- constants + resident weights ------------------------
            g1_sb = const.tile([P, 1], F32)
            nc.sync.dma_start(g1_sb, g1_d[:])
            cw1_sb = const.tile([H, CM], BF16)
            nc.sync.dma_start(cw1_sb, cw1_d[:])
            cb1_sb = const.tile([P, NCI], F32)
            nc.sync.dma_start(cb1_sb, cb1_d[:])
            cw2_sb = const.tile([P, NCI, H], BF16)
            nc.sync.dma_start(cw2_sb, cw2_d[:])
            ones_sb = const.tile([P, P], BF16)
            nc.sync.dma_start(ones_sb, ones_d[:])
            ident = const.tile([P, P], F32)
            make_identity(nc, ident)
            if nontrivial_bias1:
                bias1_sb = const.tile([P, TM], F32)
                nc.sync.dma_start(bias1_sb, bias1_d[:])
            if nontrivial_cb2:
                cb2_sb = const.tile([P, 1], F32)
                nc.sync.dma_start(cb2_sb, cb2_d[:])
                cb2_t = small.tile([P, 1], F32, tag="cb2t")
                nc.vector.tensor_copy(cb2_t, cb2_sb)
            g1_t = small.tile([P, 1], F32)
            nc.vector.tensor_copy(g1_t, g1_sb)
            eps_t = const.tile([P, 1], F32)
            nc.vector.memset(eps_t, EPS)

            # resident weights, chunked DMAs to engage all queues
            w1c_sb = const.tile([P, NC_TOK, TM], BF16)
            w2_sb = const.tile([P, KTM, NJ, 512], BF16)
            XCH = 8    # x DMA chunks per batch (of NC_TOK)
            WCH = 16   # w1c DMA chunks (of NC_TOK)
            x_sb = []
            for b in range(BL):
                xt = xall.tile([P, NC_TOK, H], BF16, tag="xall", name=f"xall{b}")
                x_sb.append(xt)
            for i in range(WCH):
                cw = NC_TOK // WCH
                nc.sync.dma_start(
                    w1c_sb[:, i * cw:(i + 1) * cw, :],
                    w1c_d[:, i * cw:(i + 1) * cw, :])
                nc.sync.dma_start(
                    w2_sb[:, :, i, :], w2_d[:, :, i, :])
                if i < XCH * BL:
                    b, q = divmod(i, XCH)
                    cq = NC_TOK // XCH
                    nc.sync.dma_start(
                        x_sb[b][:, q * cq:(q + 1) * cq, :],
                        x_d[b, :, q * cq:(q + 1) * cq, :])

            # ---- phase 1: LN1 stats + normalize + token matmul 1 -----
            rstd1 = []
            mu1 = []
            for b in range(BL):
                xt = x_sb[b]
                sums = stats.tile([P, NC_TOK], F32, tag="st_sum")
                nc.vector.tensor_reduce(
                    out=sums, in_=xt, axis=AX.X, op=ALU.add)
                sq = sqp.tile([P, NC_TOK, H], BF16, tag="sq")
                nc.vector.tensor_tensor(sq, xt, xt, ALU.mult)
                sumsq = stats.tile([P, NC_TOK], F32, tag="st_sumsq")
                nc.vector.tensor_reduce(
                    out=sumsq, in_=sq, axis=AX.X, op=ALU.add)

                mu = stats.tile([P, NC_TOK], F32, tag="st_mu")
                nc.vector.tensor_scalar_mul(mu, sums, 1.0 / H)
                ex2 = stats.tile([P, NC_TOK], F32, tag="st_ex2")
                nc.vector.tensor_scalar_mul(ex2, sumsq, 1.0 / H)
                musq = stats.tile([P, NC_TOK], F32, tag="st_musq")
                nc.vector.tensor_tensor(musq, mu, mu, ALU.mult)
                var = stats.tile([P, NC_TOK], F32, tag="st_var")
                nc.vector.tensor_tensor(var, ex2, musq, ALU.subtract)
                nc.vector.tensor_scalar(
                    out=var, in0=var, scalar1=EPS, scalar2=None, op0=ALU.add)
                std = stats.tile([P, NC_TOK], F32, tag="st_std")
                i_r = nc.scalar.activation(std, var, AF.Sqrt)
                act_phases[0].append(i_r)
                rst = stats.tile([P, NC_TOK], F32, tag="st_rstd")
                nc.vector.reciprocal_approx_fast(rst, std)
                rstd1.append(rst)
                mu1.append(mu)

            with (
                tc.tile_pool(name="ps1", bufs=BL, space="PSUM") as ps1,
                tc.tile_pool(name="pst", bufs=2, space="PSUM") as pst,
            ):
                psum1 = [ps1.tile([P, TM], F32, tag="ps1", name=f"ps1_{b}")
                         for b in range(BL)]
                for c in range(NC_TOK):
                    for b in range(BL):
                        xn = small.tile([P, P], BF16, tag="xn")
                        nc.vector.tensor_scalar(
                            out=xn,
                            in0=x_sb[b][:, c, :],
                            scalar1=mu1[b][:, c:c + 1],
                            scalar2=rstd1[b][:, c:c + 1],
                            op0=ALU.subtract,
                            op1=ALU.mult,
                        )
                        nc.tensor.matmul(
                            psum1[b],
                            xn,
                            w1c_sb[:, c, :],
                            start=(c == 0),
                            stop=(c == NC_TOK - 1),
                        )

                # ---- phase 2: token gelu, transpose, center ----------
                h1c = []  # per batch: list of KTM [P, P] bf16 tiles
                for b in range(BL):
                    h1 = small.tile([P, TM], F32, tag="h1")
                    if nontrivial_bias1:
                        nc.vector.tensor_scalar_mul(h1, psum1[b], g1_t)
                        nc.vector.tensor_add(h1, h1, bias1_sb)
                        i_g = nc.scalar.activation(h1, h1, AF.Gelu)
                    else:
                        i_g = nc.scalar.activation(h1, psum1[b], AF.Gelu,
                                                   scale=g1_t)
                    act_phases[1].append(i_g)

                    chunks = []
                    for k in range(KTM):
                        ps_t = pst.tile([P, P], F32, tag="pst")
                        nc.tensor.transpose(ps_t, h1[:, k * P:(k + 1) * P], ident)
                        h1T = small.tile([P, P], F32, tag="h1T")
                        nc.vector.tensor_copy(h1T, ps_t)
                        hsum = small.tile([P, 1], F32, tag="hsum")
                        nc.vector.tensor_reduce(
                            out=hsum, in_=h1T, axis=AX.X, op=ALU.add)
                        hmean = small.tile([P, 1], F32, tag="hmean")
                        nc.vector.tensor_scalar_mul(hmean, hsum, 1.0 / H)
                        hc = small.tile([P, P], BF16, tag="h1c")
                        nc.vector.tensor_scalar(
                            out=hc, in0=h1T, scalar1=hmean, scalar2=None,
                            op0=ALU.subtract)
                        chunks.append(hc)
                    h1c.append(chunks)

            # ---- phase 3a: token matmul 2 + LN2 stats ----------------
            with (
                tc.tile_pool(name="ps2", bufs=3, space="PSUM") as ps2,
                tc.tile_pool(name="psv", bufs=2, space="PSUM") as psv,
            ):
                y2n = []
                for b in range(BL):
                    y2n.append(xall.tile([P, N], BF16, tag="xall", name=f"y2n{b}"))

                for j in range(NJ):
                    for b in range(BL):
                        p2 = ps2.tile([P, 512], F32, tag="ps2")
                        for k in range(KTM):
                            nc.tensor.matmul(
                                p2, h1c[b][k], w2_sb[:, k, j, :],
                                start=(k == 0), stop=(k == KTM - 1))
                        sq2 = sq2p.tile([P, 512], BF16, tag="sq2")
                        i_s = nc.scalar.activation(
                            sq2, p2, AF.Square, scale=float(1.0 / np.sqrt(H)))
                        act_phases[2].append(i_s)
                        vps = psv.tile([P, 512], F32, tag="psv")
                        nc.tensor.matmul(vps, ones_sb, sq2, start=True, stop=True)
                        std = rstdp.tile([P, 512], F32, tag="std")
                        i_r = nc.scalar.activation(std, vps, AF.Sqrt, bias=eps_t)
                        act_phases[2].append(i_r)
                        rstd = rstdp.tile([P, 512], F32, tag="rstd")
                        nc.vector.reciprocal_approx_fast(rstd, std)
                        nc.vector.tensor_tensor(
                            y2n[b][:, j * 512:(j + 1) * 512],
                            p2, rstd, ALU.mult)

            # ---- phase 3b: channel MLP -------------------------------
            with (
                tc.tile_pool(name="psr", bufs=1, space="PSUM") as psr,
                tc.tile_pool(name="pso", bufs=2, space="PSUM") as pso,
            ):
                for j in range(NJ):
                    for b in range(BL):
                        y2s = y2n[b][:, j * 512:(j + 1) * 512]
                        raw2 = psr.tile([P, NCI * 512], F32, tag="psr")
                        for ci in range(NCI):
                            nc.tensor.matmul(
                                raw2[:, ci * 512:(ci + 1) * 512],
                                cw1_sb[:, ci * P:(ci + 1) * P],
                                y2s, start=True, stop=True)
                        g2 = g2p.tile([P, NCI * 512], BF16, tag="g2")
                        if nontrivial_bias1:
                            # general path: per-ci bias
                            for ci in range(NCI):
                                i_g = nc.scalar.activation(
                                    g2[:, ci * 512:(ci + 1) * 512],
                                    raw2[:, ci * 512:(ci + 1) * 512],
                                    AF.Gelu, bias=cb1_sb[:, ci:ci + 1])
                                act_phases[3].append(i_g)
                        else:
                            i_g = nc.scalar.activation(g2, raw2, AF.Gelu)
                            act_phases[3].append(i_g)

                        po = pso.tile([P, 512], F32, tag="pso")
                        for ci in range(NCI):
                            nc.tensor.matmul(
                                po,
                                cw2_sb[:, ci, :],
                                g2[:, ci * 512:(ci + 1) * 512],
                                start=(ci == 0), stop=(ci == NCI - 1))
                        osb = outp.tile([P, 512], BF16, tag="osb")
                        if nontrivial_cb2:
                            nc.vector.tensor_scalar(
                                out=osb, in0=po, scalar1=cb2_t, scalar2=None,
                                op0=ALU.add)
                        else:
                            nc.vector.tensor_copy(osb, po)
                        nc.sync.dma_start(
                            out_v[b][:, j * 512:(j + 1) * 512], osb)

            # ---- ACT table-set ordering edges ------------------------
            for ph in range(3):
                for f in act_phases[ph + 1]:
                    for t in act_phases[ph]:
                        bass_rust.add_dep_helper(
                            f.ins, t.ins, sync=False,
                            reason="act table set phase ordering")

    nc.compile()
    return nc


def _host_prep(inputs):
    import ml_dtypes
    BF = ml_dtypes.bfloat16

    x = np.asarray(inputs["x"], dtype=np.float32)
    ln1_g = np.asarray(inputs["ln1_g"], np.float32)
    ln1_b = np.asarray(inputs["ln1_b"], np.float32)
    ln2_g = np.asarray(inputs["ln2_g"], np.float32)
    ln2_b = np.asarray(inputs["ln2_b"], np.float32)
    tok_w1 = np.asarray(inputs["tok_w1"], np.float32)
    tok_b1 = np.asarray(inputs["tok_b1"], np.float32)
    tok_w2 = np.asarray(inputs["tok_w2"], np.float32)
    ch_w1 = np.asarray(inputs["ch_w1"], np.float32)
    ch_b1 = np.asarray(inputs["ch_b1"], np.float32)
    ch_w2 = np.asarray(inputs["ch_w2"], np.float32)
    ch_b2 = np.asarray(inputs["ch_b2"], np.float32)

    w1c = np.cumsum(tok_w1, axis=0, dtype=np.float64).astype(np.float32)
    colsum1 = w1c.sum(axis=0, dtype=np.float64).astype(np.float32)
    bias1 = ln1_b[:, None] * colsum1[None, :] + tok_b1[None, :]
    cw1 = (ln2_g[:, None] * ch_w1).astype(np.float32)
    cb1 = (ch_b1 + ch_w1.T @ ln2_b).astype(np.float32)

    nontrivial_bias1 = bool(np.any(bias1 != 0.0) or np.any(cb1 != 0.0))
    nontrivial_cb2 = bool(np.any(ch_b2 != 0.0))

    # device layouts (see _build comments)
    x_dev = np.ascontiguousarray(
        x.reshape(B, NC_TOK, P, H).transpose(0, 2, 1, 3)).astype(BF)
    w1c_dev = np.ascontiguousarray(
        w1c.reshape(NC_TOK, P, TM).transpose(1, 0, 2)).astype(BF)
    w2_dev = np.ascontiguousarray(
        tok_w2.reshape(KTM, P, NJ, 512).transpose(1, 0, 2, 3)).astype(BF)
    cw2_dev = np.ascontiguousarray(
        ch_w2.reshape(NCI, P, H).transpose(1, 0, 2)).astype(BF)

    shared = {
        "w1c": w1c_dev,
        "w2": w2_dev,
        "g1": ln1_g.reshape(P, 1).copy(),
        "bias1": np.ascontiguousarray(bias1, np.float32),
        "cw1": cw1.astype(BF),
        "cb1": np.ascontiguousarray(cb1.reshape(NCI, P).T.copy()),
        "cw2": cw2_dev,
        "cb2": ch_b2.reshape(P, 1).astype(np.float32).copy(),
        "ones": np.ones((P, P), BF),
    }
    return x_dev, shared, nontrivial_bias1, nontrivial_cb2


def _assemble(r):
    return r["out"].astype(np.float32).transpose(0, 2, 1)


def kernel(**inputs) -> np.ndarray:
    from concourse.bass_utils import run_bass_kernel_spmd

    x, shared, nb1, nb2 = _host_prep(inputs)

    key = (nb1, nb2)
    if key not in _cached:
        _cached[key] = _build(nb1, nb2)
    nc = _cached[key]

    in_maps = []
    for c in range(NCORES):
        m = dict(shared)
        m["x"] = np.ascontiguousarray(x[c * BL:(c + 1) * BL])
        in_maps.append(m)

    res = run_bass_kernel_spmd(nc, in_maps, core_ids=list(range(NCORES)))
    out = np.concatenate([_assemble(r) for r in res.results], axis=0)
    return np.ascontiguousarray(out, dtype=np.float32)


if __name__ == "__main__":
    rng = np.random.default_rng(0)
    ins = {
        "x": rng.standard_normal((B, N, H)).astype(np.float32),
        "ln1_g": np.ones(H, np.float32),
        "ln1_b": np.zeros(H, np.float32),
        "ln2_g": np.ones(H, np.float32),
        "ln2_b": np.zeros(H, np.float32),
        "tok_w1": (rng.standard_normal((N, TM)) * 0.02).astype(np.float32),
        "tok_b1": np.zeros(TM, np.float32),
        "tok_w2": (rng.standard_normal((TM, N)) * 0.02).astype(np.float32),
        "tok_b2": np.zeros(N, np.float32),
        "ch_w1": (rng.standard_normal((H, CM)) * 0.02).astype(np.float32),
        "ch_b1": np.zeros(CM, np.float32),
        "ch_w2": (rng.standard_normal((CM, H)) * 0.02).astype(np.float32),
        "ch_b2": np.zeros(H, np.float32),
    }
    out = kernel(**ins)
    print("out", out.shape, out.dtype)


# revision 5
# speedup vs baseline: 1.0896x; 1.0458x over previous
"""Trainium2 Bass kernel for nn_AutoregressiveMixerBlock.

Reference computation (per batch b):
  y  = LN_H(x)                                    # layer norm over H
  t  = revcumsum_N(y)                             # t[j] = sum_{i>=j} y[i]
  h  = gelu(t^T @ tok_w1 + tok_b1)                # [H, TM]
  y2 = (h @ tok_w2 + tok_b2)^T                    # [N, H]
  y3 = LN_H(y2)
  out = gelu(y3 @ ch_w1 + ch_b1) @ ch_w2 + ch_b2  # [N, H]

Key algebraic folds (exact in real arithmetic, applied on host):
  * revcumsum+matmul:  sum_j t[j,h] w1[j,m] = sum_i y[i,h] W1c[i,m]
    with W1c = cumsum(tok_w1, axis=0) -> no on-device cumsum at all.
  * LN1 gain/bias move past the token matmul.
  * tok_b2 and the LN2 mean both vanish by centering h^T by its
    per-row (over H) mean before the second token matmul.
  * LN2 gain/bias fold into ch_w1 / ch_b1.

Perf structure:
  * every matmul operand is bf16.
  * x / w1c / w2 stream in as independent chunk tiles so compute
    starts as soon as the first chunks land (no whole-tensor DMA
    barrier); x chunks are issued first (LN1 is the critical path).
  * channel-MLP hidden runs in two [P,1024] halves, double-buffered,
    so the PE never stalls on the gelu of the previous iteration.
  * LN2 sqrt/reciprocal are batched in [P,1024] pairs.
  * output staging copies run on the idle GpSimd (Pool) engine.
  * output is written bf16 and converted on host.

Sharding: data-parallel over B across 8 cores (2 batches per core),
weights replicated.
"""

import numpy as np

B, N, H = 16, 8192, 128
TM, CM = 256, 512
EPS = 1e-5
NCORES = 8
BL = B // NCORES          # batches per core
P = 128                   # partitions
NC_TOK = N // P           # 64 token chunks of 128
NJ = N // 512             # 16 column chunks of 512
KTM = TM // P             # 2 k-chunks for the second token matmul
NCI = CM // P             # 4 chunks of the channel hidden dim
XQ = 8                    # x DMA/stat chunks per batch (of NC_TOK)
WQ = 16                   # w1c chunk tiles

_cached = {}


def _build(nontrivial_bias1, nontrivial_cb2):
    import concourse.bass as bass
    import concourse.mybir as mybir
    import concourse.tile as tile
    from concourse import bacc
    from concourse.masks import make_identity
    import bass_rust

    F32 = mybir.dt.float32
    BF16 = mybir.dt.bfloat16
    AF = mybir.ActivationFunctionType
    ALU = mybir.AluOpType
    AX = mybir.AxisListType

    nc = bacc.Bacc()

    # ---- DRAM tensors -------------------------------------------------
    x_d = nc.dram_tensor("x", [BL, P, NC_TOK, H], BF16, kind="ExternalInput")
    w1c_d = nc.dram_tensor("w1c", [P, NC_TOK, TM], BF16, kind="ExternalInput")
    w2_d = nc.dram_tensor("w2", [P, KTM, NJ, 512], BF16, kind="ExternalInput")
    g1_d = nc.dram_tensor("g1", [P, 1], F32, kind="ExternalInput")
    bias1_d = nc.dram_tensor("bias1", [P, TM], F32, kind="ExternalInput")
    cw1_d = nc.dram_tensor("cw1", [H, CM], BF16, kind="ExternalInput")
    cb1_d = nc.dram_tensor("cb1", [P, NCI], F32, kind="ExternalInput")
    cw2_d = nc.dram_tensor("cw2", [P, NCI, H], BF16, kind="ExternalInput")
    cb2_d = nc.dram_tensor("cb2", [P, 1], F32, kind="ExternalInput")
    ones_d = nc.dram_tensor("ones", [P, P], BF16, kind="ExternalInput")
    out_d = nc.dram_tensor("out", [BL, H, N], BF16, kind="ExternalOutput")

    out_v = [out_d[b] for b in range(BL)]
    XC = NC_TOK // XQ     # token chunks per x chunk (8)
    WC = NC_TOK // WQ     # token chunks per w1c chunk (4)

    act_phases = [[], [], [], []]  # ACT table-set phase buckets

    with tile.TileContext(nc) as tc:
        import contextlib
        with contextlib.ExitStack() as ctx:
            const = ctx.enter_context(tc.tile_pool(name="const", bufs=1))
            xall = ctx.enter_context(tc.tile_pool(name="xall", bufs=BL * XQ))
            y2p = ctx.enter_context(tc.tile_pool(name="y2p", bufs=BL))
            w1p = ctx.enter_context(tc.tile_pool(name="w1p", bufs=WQ))
            w2p = ctx.enter_context(tc.tile_pool(name="w2p", bufs=NJ))
            stats = ctx.enter_context(tc.tile_pool(name="stats", bufs=2 * BL * XQ))
            small = ctx.enter_context(tc.tile_pool(name="small", bufs=4))
            sqp = ctx.enter_context(tc.tile_pool(name="sqp", bufs=4))
            sq2p = ctx.enter_context(tc.tile_pool(name="sq2p", bufs=3))
            rstdp = ctx.enter_context(tc.tile_pool(name="rstdp", bufs=3))
            g2p = ctx.enter_context(tc.tile_pool(name="g2p", bufs=4))
            outp = ctx.enter_context(tc.tile_pool(name="outp", bufs=3))

            # ---- x DMAs first (LN1 is the startup critical path) -----
            x_sb = []     # [b][q] -> [P, XC, H] tile
            for b in range(BL):
                x_sb.append([])
                for q in range(XQ):
                    xt = xall.tile([P, XC, H], BF16, tag="xall",
                                   name=f"x{b}_{q}")
                    x_sb[b].append(xt)
            for q in range(XQ):
                for b in range(BL):
                    nc.sync.dma_start(
                        x_sb[b][q], x_d[b, :, q * XC:(q + 1) * XC, :])

            # ---- weight chunk DMAs -----------------------------------
            w1_sb = []
            for i in range(WQ):
                wt = w1p.tile([P, WC, TM], BF16, tag="w1", name=f"w1_{i}")
                nc.sync.dma_start(wt, w1c_d[:, i * WC:(i + 1) * WC, :])
                w1_sb.append(wt)
            w2_sb = []
            for j in range(NJ):
                wt = w2p.tile([P, KTM, 512], BF16, tag="w2", name=f"w2_{j}")
                nc.sync.dma_start(wt, w2_d[:, :, j, :])
                w2_sb.append(wt)

            # ---- constants -------------------------------------------
            g1_sb = const.tile([P, 1], F32)
            nc.sync.dma_start(g1_sb, g1_d[:])
            cw1_sb = const.tile([H, CM], BF16)
            nc.sync.dma_start(cw1_sb, cw1_d[:])
            cb1_sb = const.tile([P, NCI], F32)
            nc.sync.dma_start(cb1_sb, cb1_d[:])
            cw2_sb = const.tile([P, NCI, H], BF16)
            nc.sync.dma_start(cw2_sb, cw2_d[:])
            ones_sb = const.tile([P, P], BF16)
            nc.sync.dma_start(ones_sb, ones_d[:])
            ident = const.tile([P, P], F32)
            make_identity(nc, ident)
            if nontrivial_bias1:
                bias1_sb = const.tile([P, TM], F32)
                nc.sync.dma_start(bias1_sb, bias1_d[:])
            if nontrivial_cb2:
                cb2_sb = const.tile([P, 1], F32)
                nc.sync.dma_start(cb2_sb, cb2_d[:])
                cb2_t = small.tile([P, 1], F32, tag="cb2t")
                nc.vector.tensor_copy(cb2_t, cb2_sb)
            g1_t = small.tile([P, 1], F32)
            nc.vector.tensor_copy(g1_t, g1_sb)
            eps_t = const.tile([P, 1], F32)
            nc.vector.memset(eps_t, EPS)

            # ---- phase 1: LN1 stats per chunk ------------------------
            rstd1 = []   # [b][q] -> [P, XC]
            mu1 = []
            for b in range(BL):
                rstd1.append([])
                mu1.append([])
                for q in range(XQ):
                    xt = x_sb[b][q]
                    sums = stats.tile([P, XC], F32, tag="st_sum")
                    nc.vector.tensor_reduce(
                        out=sums, in_=xt, axis=AX.X, op=ALU.add)
                    sq = sqp.tile([P, XC, H], BF16, tag="sq")
                    nc.vector.tensor_tensor(sq, xt, xt, ALU.mult)
                    sumsq = stats.tile([P, XC], F32, tag="st_sumsq")
                    nc.vector.tensor_reduce(
                        out=sumsq, in_=sq, axis=AX.X, op=ALU.add)

                    mu = stats.tile([P, XC], F32, tag="st_mu")
                    nc.vector.tensor_scalar_mul(mu, sums, 1.0 / H)
                    ex2 = stats.tile([P, XC], F32, tag="st_ex2")
                    nc.vector.tensor_scalar_mul(ex2, sumsq, 1.0 / H)
                    var = stats.tile([P, XC], F32, tag="st_var")
                    nc.vector.tensor_tensor(var, mu, mu, ALU.mult)
                    nc.vector.tensor_tensor(var, ex2, var, ALU.subtract)
                    nc.vector.tensor_scalar(
                        out=var, in0=var, scalar1=EPS, scalar2=None,
                        op0=ALU.add)
                    std = stats.tile([P, XC], F32, tag="st_std")
                    i_r = nc.scalar.activation(std, var, AF.Sqrt)
                    act_phases[0].append(i_r)
                    rst = stats.tile([P, XC], F32, tag="st_rstd")
                    nc.vector.reciprocal_approx_fast(rst, std)
                    rstd1[b].append(rst)
                    mu1[b].append(mu)

            # ---- token matmul 1 --------------------------------------
            with (
                tc.tile_pool(name="ps1", bufs=BL, space="PSUM") as ps1,
                tc.tile_pool(name="pst", bufs=2, space="PSUM") as pst,
            ):
                psum1 = [ps1.tile([P, TM], F32, tag="ps1", name=f"ps1_{b}")
                         for b in range(BL)]
                for c in range(NC_TOK):
                    q, r = divmod(c, XC)
                    wq, wr = divmod(c, WC)
                    for b in range(BL):
                        xn = small.tile([P, P], BF16, tag="xn")
                        eng = nc.vector if (c % 2 == 0) else nc.gpsimd
                        eng.tensor_scalar(
                            out=xn,
                            in0=x_sb[b][q][:, r, :],
                            scalar1=mu1[b][q][:, r:r + 1],
                            scalar2=rstd1[b][q][:, r:r + 1],
                            op0=ALU.subtract,
                            op1=ALU.mult,
                        )
                        nc.tensor.matmul(
                            psum1[b],
                            xn,
                            w1_sb[wq][:, wr, :],
                            start=(c == 0),
                            stop=(c == NC_TOK - 1),
                        )

                # ---- phase 2: token gelu, transpose, center ----------
                h1c = []  # per batch: list of KTM [P, P] bf16 tiles
                for b in range(BL):
                    h1 = small.tile([P, TM], F32, tag="h1")
                    if nontrivial_bias1:
                        nc.vector.tensor_scalar_mul(h1, psum1[b], g1_t)
                        nc.vector.tensor_add(h1, h1, bias1_sb)
                        i_g = nc.scalar.activation(h1, h1, AF.Gelu)
                    else:
                        i_g = nc.scalar.activation(h1, psum1[b], AF.Gelu,
                                                   scale=g1_t)
                    act_phases[1].append(i_g)

                    chunks = []
                    for k in range(KTM):
                        ps_t = pst.tile([P, P], F32, tag="pst")
                        nc.tensor.transpose(ps_t, h1[:, k * P:(k + 1) * P], ident)
                        h1T = small.tile([P, P], F32, tag="h1T")
                        nc.vector.tensor_copy(h1T, ps_t)
                        hsum = small.tile([P, 1], F32, tag="hsum")
                        nc.vector.tensor_reduce(
                            out=hsum, in_=h1T, axis=AX.X, op=ALU.add)
                        hmean = small.tile([P, 1], F32, tag="hmean")
                        nc.vector.tensor_scalar_mul(hmean, hsum, 1.0 / H)
                        hc = small.tile([P, P], BF16, tag="h1c")
                        nc.vector.tensor_scalar(
                            out=hc, in0=h1T, scalar1=hmean, scalar2=None,
                            op0=ALU.subtract)
                        chunks.append(hc)
                    h1c.append(chunks)

            # ---- phase 3a: token matmul 2 + LN2 stats ----------------
            # iterations idx = 2*j + b; sqrt/recip batched per pair.
            with (
                tc.tile_pool(name="ps2", bufs=4, space="PSUM") as ps2,
                tc.tile_pool(name="psv", bufs=2, space="PSUM") as psv,
            ):
                y2n = []
                for b in range(BL):
                    y2n.append(y2p.tile([P, N], BF16, tag="y2", name=f"y2n{b}"))

                p2s = {}
                vpair = None
                for j in range(NJ):
                    for b in range(BL):
                        idx = 2 * j + b
                        half = idx % 2
                        p2 = ps2.tile([P, 512], F32, tag="ps2")
                        p2s[idx] = p2
                        for k in range(KTM):
                            nc.tensor.matmul(
                                p2, h1c[b][k], w2_sb[j][:, k, :],
                                start=(k == 0), stop=(k == KTM - 1))
                        sq2 = sq2p.tile([P, 512], BF16, tag="sq2")
                        i_s = nc.scalar.activation(
                            sq2, p2, AF.Square, scale=float(1.0 / np.sqrt(H)))
                        act_phases[2].append(i_s)
                        if half == 0:
                            vpair = psv.tile([P, 1024], F32, tag="psv")
                        nc.tensor.matmul(
                            vpair[:, half * 512:(half + 1) * 512],
                            ones_sb, sq2, start=True, stop=True)
                        if half == 1:
                            std = rstdp.tile([P, 1024], F32, tag="std")
                            i_r = nc.scalar.activation(
                                std, vpair, AF.Sqrt, bias=eps_t)
                            act_phases[2].append(i_r)
                            rstd = rstdp.tile([P, 1024], F32, tag="rstd")
                            nc.vector.reciprocal_approx_fast(rstd, std)
                            for h2 in range(2):
                                i2 = idx - 1 + h2
                                j2, b2 = divmod(i2, 2)
                                nc.vector.tensor_tensor(
                                    y2n[b2][:, j2 * 512:(j2 + 1) * 512],
                                    p2s.pop(i2),
                                    rstd[:, h2 * 512:(h2 + 1) * 512],
                                    ALU.mult)

            # ---- phase 3b: channel MLP (half-hidden pipelining) ------
            with (
                tc.tile_pool(name="psr", bufs=3, space="PSUM") as psr,
                tc.tile_pool(name="pso", bufs=2, space="PSUM") as pso,
            ):
                for j in range(NJ):
                    for b in range(BL):
                        y2s = y2n[b][:, j * 512:(j + 1) * 512]
                        g2 = g2p.tile([P, NCI, 512], BF16, tag="g2")
                        for hh in range(2):
                            raw2 = psr.tile([P, 2, 512], F32, tag="psr")
                            for ci2 in range(2):
                                ci = hh * 2 + ci2
                                nc.tensor.matmul(
                                    raw2[:, ci2, :],
                                    cw1_sb[:, ci * P:(ci + 1) * P],
                                    y2s, start=True, stop=True)
                            if nontrivial_bias1:
                                for ci2 in range(2):
                                    ci = hh * 2 + ci2
                                    i_g = nc.scalar.activation(
                                        g2[:, ci, :],
                                        raw2[:, ci2, :],
                                        AF.Gelu, bias=cb1_sb[:, ci:ci + 1])
                                    act_phases[3].append(i_g)
                            else:
                                i_g = nc.scalar.activation(
                                    g2[:, hh * 2:(hh + 1) * 2, :], raw2,
                                    AF.Gelu)
                                act_phases[3].append(i_g)

                        po = pso.tile([P, 512], F32, tag="pso")
                        for ci in range(NCI):
                            nc.tensor.matmul(
                                po,
                                cw2_sb[:, ci, :],
                                g2[:, ci, :],
                                start=(ci == 0), stop=(ci == NCI - 1))
                        osb = outp.tile([P, 512], BF16, tag="osb")
                        if nontrivial_cb2:
                            nc.vector.tensor_scalar(
                                out=osb, in0=po, scalar1=cb2_t, scalar2=None,
                                op0=ALU.add)
                        else:
                            nc.vector.tensor_copy(osb, po)
                        nc.sync.dma_start(
                            out_v[b][:, j * 512:(j + 1) * 512], osb)

            # ---- ACT table-set ordering edges ------------------------
            for ph in range(3):
                for f in act_phases[ph + 1]:
                    for t in act_phases[ph]:
                        bass_rust.add_dep_helper(
                            f.ins, t.ins, sync=False,
                            reason="act table set phase ordering")

    nc.compile()
    return nc


def _host_prep(inputs):
    import ml_dtypes
    BF = ml_dtypes.bfloat16

    x = np.asarray(inputs["x"], dtype=np.float32)
    ln1_g = np.asarray(inputs["ln1_g"], np.float32)
    ln1_b = np.asarray(inputs["ln1_b"], np.float32)
    ln2_g = np.asarray(inputs["ln2_g"], np.float32)
    ln2_b = np.asarray(inputs["ln2_b"], np.float32)
    tok_w1 = np.asarray(inputs["tok_w1"], np.float32)
    tok_b1 = np.asarray(inputs["tok_b1"], np.float32)
    tok_w2 = np.asarray(inputs["tok_w2"], np.float32)
    ch_w1 = np.asarray(inputs["ch_w1"], np.float32)
    ch_b1 = np.asarray(inputs["ch_b1"], np.float32)
    ch_w2 = np.asarray(inputs["ch_w2"], np.float32)
    ch_b2 = np.asarray(inputs["ch_b2"], np.float32)

    w1c = np.cumsum(tok_w1, axis=0, dtype=np.float64).astype(np.float32)
    colsum1 = w1c.sum(axis=0, dtype=np.float64).astype(np.float32)
    bias1 = ln1_b[:, None] * colsum1[None, :] + tok_b1[None, :]
    cw1 = (ln2_g[:, None] * ch_w1).astype(np.float32)
    cb1 = (ch_b1 + ch_w1.T @ ln2_b).astype(np.float32)

    nontrivial_bias1 = bool(np.any(bias1 != 0.0) or np.any(cb1 != 0.0))
    nontrivial_cb2 = bool(np.any(ch_b2 != 0.0))

    x_dev = np.ascontiguousarray(
        x.reshape(B, NC_TOK, P, H).transpose(0, 2, 1, 3)).astype(BF)
    w1c_dev = np.ascontiguousarray(
        w1c.reshape(NC_TOK, P, TM).transpose(1, 0, 2)).astype(BF)
    w2_dev = np.ascontiguousarray(
        tok_w2.reshape(KTM, P, NJ, 512).transpose(1, 0, 2, 3)).astype(BF)
    cw2_dev = np.ascontiguousarray(
        ch_w2.reshape(NCI, P, H).transpose(1, 0, 2)).astype(BF)

    shared = {
        "w1c": w1c_dev,
        "w2": w2_dev,
        "g1": ln1_g.reshape(P, 1).copy(),
        "bias1": np.ascontiguousarray(bias1, np.float32),
        "cw1": cw1.astype(BF),
        "cb1": np.ascontiguousarray(cb1.reshape(NCI, P).T.copy()),
        "cw2": cw2_dev,
        "cb2": ch_b2.reshape(P, 1).astype(np.float32).copy(),
        "ones": np.ones((P, P), BF),
    }
    return x_dev, shared, nontrivial_bias1, nontrivial_cb2


def _assemble(r):
    return r["out"].astype(np.float32).transpose(0, 2, 1)


def kernel(**inputs) -> np.ndarray:
    from concourse.bass_utils import run_bass_kernel_spmd

    x, shared, nb1, nb2 = _host_prep(inputs)

    key = (nb1, nb2)
    if key not in _cached:
        _cached[key] = _build(nb1, nb2)
    nc = _cached[key]

    in_maps = []
    for c in range(NCORES):
        m = dict(shared)
        m["x"] = np.ascontiguousarray(x[c * BL:(c + 1) * BL])
        in_maps.append(m)

    res = run_bass_kernel_spmd(nc, in_maps, core_ids=list(range(NCORES)))
    out = np.concatenate([_assemble(r) for r in res.results], axis=0)
    return np.ascontiguousarray(out, dtype=np.float32)


if __name__ == "__main__":
    rng = np.random.default_rng(0)
    ins = {
        "x": rng.standard_normal((B, N, H)).astype(np.float32),
        "ln1_g": np.ones(H, np.float32),
        "ln1_b": np.zeros(H, np.float32),
        "ln2_g": np.ones(H, np.float32),
        "ln2_b": np.zeros(H, np.float32),
        "tok_w1": (rng.standard_normal((N, TM)) * 0.02).astype(np.float32),
        "tok_b1": np.zeros(TM, np.float32),
        "tok_w2": (rng.standard_normal((TM, N)) * 0.02).astype(np.float32),
        "tok_b2": np.zeros(N, np.float32),
        "ch_w1": (rng.standard_normal((H, CM)) * 0.02).astype(np.float32),
        "ch_b1": np.zeros(CM, np.float32),
        "ch_w2": (rng.standard_normal((CM, H)) * 0.02).astype(np.float32),
        "ch_b2": np.zeros(H, np.float32),
    }
    out = kernel(**ins)
    print("out", out.shape, out.dtype)


# revision 16
# speedup vs baseline: 1.5206x; 1.3956x over previous
"""Trainium2 Bass kernel for nn_AutoregressiveMixerBlock.

Reference computation (per batch b):
  y  = LN_H(x)                                    # layer norm over H
  t  = revcumsum_N(y)                             # t[j] = sum_{i>=j} y[i]
  h  = gelu(t^T @ tok_w1 + tok_b1)                # [H, TM]
  y2 = (h @ tok_w2 + tok_b2)^T                    # [N, H]
  y3 = LN_H(y2)
  out = gelu(y3 @ ch_w1 + ch_b1) @ ch_w2 + ch_b2  # [N, H]

Key algebraic folds (exact in real arithmetic, applied on host):
  * revcumsum+matmul:  sum_j t[j,h] w1[j,m] = sum_i y[i,h] W1c[i,m]
    with W1c = cumsum(tok_w1, axis=0) -> no on-device cumsum at all.
  * LN1 gain/bias move past the token matmul.
  * tok_b2 and the LN2 mean both vanish by centering h^T by its
    per-row (over H) mean before the second token matmul.
  * LN2 gain/bias fold into ch_w1 / ch_b1.

Perf structure:
  * fp16 matmul operands everywhere (bf16 speed, 8x the mantissa;
    fp8 DoubleRow was tried and its quantization noise alone exceeds
    the 2e-2 error budget).
  * x / w1c / w2 stream in as independent chunk tiles; x first.
  * LN1 stats emission is interleaved with the token matmul per
    stats group, so the PE starts as soon as the first chunks land.
  * the LN1 normalize runs as two broadcast tensor_tensor passes per
    x chunk (xn = x*rstd - mu*rstd) instead of 128 tiny ops.
  * channel-MLP hidden runs in two [P,1024] halves, triple-buffered.
  * LN2 sqrt/reciprocal are batched in [P,1024] pairs; the output
    staging copy alternates between DVE and ACT.
  * output is written fp16 and converted on host.

Sharding: data-parallel over B across 8 cores (2 batches per core),
weights replicated.
"""

import numpy as np

B, N, H = 16, 8192, 128
TM, CM = 256, 512
EPS = 1e-5
NCORES = 8
BL = B // NCORES          # batches per core
P = 128                   # partitions
NC_TOK = N // P           # 64 token chunks of 128
NJ = N // 512             # 16 column chunks of 512
KTM = TM // P             # 2 k-chunks for the second token matmul
NCI = CM // P             # 4 chunks of the channel hidden dim
XQ = 8                    # x DMA/stat chunks per batch (of NC_TOK)
WQ = 16                   # w1c chunk tiles
SGRP = 2                  # x chunks per stats-combine group

_cached = {}


def _build(nontrivial_bias1, nontrivial_cb2):
    import concourse.bass as bass
    import concourse.mybir as mybir
    import concourse.tile as tile
    from concourse import bacc
    from concourse.masks import make_identity
    import bass_rust

    F32 = mybir.dt.float32
    FP16 = mybir.dt.float16
    AF = mybir.ActivationFunctionType
    ALU = mybir.AluOpType
    AX = mybir.AxisListType

    nc = bacc.Bacc()

    # ---- DRAM tensors -------------------------------------------------
    x_d = nc.dram_tensor("x", [BL, P, NC_TOK, H], FP16, kind="ExternalInput")
    w1c_d = nc.dram_tensor("w1c", [P, NC_TOK, TM], FP16, kind="ExternalInput")
    w2_d = nc.dram_tensor("w2", [P, KTM, NJ, 512], FP16, kind="ExternalInput")
    g1_d = nc.dram_tensor("g1", [P, 1], F32, kind="ExternalInput")
    bias1_d = nc.dram_tensor("bias1", [P, TM], F32, kind="ExternalInput")
    cw1_d = nc.dram_tensor("cw1", [H, CM], FP16, kind="ExternalInput")
    cb1_d = nc.dram_tensor("cb1", [P, NCI], F32, kind="ExternalInput")
    cw2_d = nc.dram_tensor("cw2", [P, NCI, H], FP16, kind="ExternalInput")
    cb2_d = nc.dram_tensor("cb2", [P, 1], F32, kind="ExternalInput")
    ones_d = nc.dram_tensor("ones", [P, P], FP16, kind="ExternalInput")
    out_d = nc.dram_tensor("out", [BL, H, N], FP16, kind="ExternalOutput")

    out_v = [out_d[b] for b in range(BL)]
    XC = NC_TOK // XQ     # token chunks per x chunk (8)
    WC = NC_TOK // WQ     # token chunks per w1c chunk (4)
    GC = XC * SGRP        # token chunks per stats group (16)
    NG = XQ // SGRP       # stats groups per batch (4)

    act_phases = [[], [], [], []]  # ACT table-set phase buckets

    with tile.TileContext(nc) as tc:
        import contextlib
        with contextlib.ExitStack() as ctx:
            const = ctx.enter_context(tc.tile_pool(name="const", bufs=1))
            y2p = ctx.enter_context(tc.tile_pool(name="y2p", bufs=BL))
            w2p = ctx.enter_context(tc.tile_pool(name="w2p", bufs=NJ))
            small = ctx.enter_context(tc.tile_pool(name="small", bufs=6))
            outp = ctx.enter_context(tc.tile_pool(name="outp", bufs=3))
            # phase-1-scoped pools: their SBUF is recycled for phase 3
            ph1 = contextlib.ExitStack()
            xall = ph1.enter_context(tc.tile_pool(name="xall", bufs=BL * XQ))
            w1p = ph1.enter_context(tc.tile_pool(name="w1p", bufs=WQ))
            stats = ph1.enter_context(
                tc.tile_pool(name="stats", bufs=2 * BL * NG))
            sqp = ph1.enter_context(tc.tile_pool(name="sqp", bufs=4))
            xnp = ph1.enter_context(tc.tile_pool(name="xnp", bufs=8))

            # ---- x DMAs first (LN1 is the startup critical path) -----
            x_sb = []     # [b][q] -> [P, XC, H] tile
            for b in range(BL):
                x_sb.append([])
                for q in range(XQ):
                    xt = xall.tile([P, XC, H], FP16, tag="xall",
                                   name=f"x{b}_{q}")
                    x_sb[b].append(xt)
            w1_sb = [None] * WQ
            for q in range(XQ):
                for b in range(BL):
                    nc.sync.dma_start(
                        x_sb[b][q], x_d[b, :, q * XC:(q + 1) * XC, :])
                for i in (2 * q, 2 * q + 1):
                    wt = w1p.tile([P, WC, TM], FP16, tag="w1", name=f"w1_{i}")
                    nc.sync.dma_start(wt, w1c_d[:, i * WC:(i + 1) * WC, :])
                    w1_sb[i] = wt
            w2_sb = []
            for j in range(NJ):
                wt = w2p.tile([P, KTM, 512], FP16, tag="w2", name=f"w2_{j}")
                nc.sync.dma_start(wt, w2_d[:, :, j, :])
                w2_sb.append(wt)

            # ---- constants -------------------------------------------
            g1_sb = const.tile([P, 1], F32)
            nc.sync.dma_start(g1_sb, g1_d[:])
            cw1_sb = const.tile([H, CM], FP16)
            nc.sync.dma_start(cw1_sb, cw1_d[:])
            cb1_sb = const.tile([P, NCI], F32)
            nc.sync.dma_start(cb1_sb, cb1_d[:])
            cw2_sb = const.tile([P, NCI, H], FP16)
            nc.sync.dma_start(cw2_sb, cw2_d[:])
            ones_sb = const.tile([P, P], FP16)
            nc.sync.dma_start(ones_sb, ones_d[:])
            ident = const.tile([P, P], F32)
            make_identity(nc, ident)
            if nontrivial_bias1:
                bias1_sb = const.tile([P, TM], F32)
                nc.sync.dma_start(bias1_sb, bias1_d[:])
            if nontrivial_cb2:
                cb2_sb = const.tile([P, 1], F32)
                nc.sync.dma_start(cb2_sb, cb2_d[:])
                cb2_t = small.tile([P, 1], F32, tag="cb2t")
                nc.vector.tensor_copy(cb2_t, cb2_sb)
            g1_t = small.tile([P, 1], F32)
            nc.vector.tensor_copy(g1_t, g1_sb)
            eps_t = const.tile([P, 1], F32)
            nc.vector.memset(eps_t, EPS)

            # ---- phase 1: LN1 + token matmul 1, group-interleaved ----
            def emit_stats(b, g):
                """LN1 stats for chunk group g of batch b ->
                (rstd [P,GC], mur=mu*rstd [P,GC])."""
                sums = stats.tile([P, GC], FP16, tag="sums")
                ssq = stats.tile([P, GC], FP16, tag="ssq")
                for q in range(SGRP):
                    xt = x_sb[b][g * SGRP + q]
                    sl = slice(q * XC, (q + 1) * XC)
                    with nc.allow_low_precision(
                            reason="fp16 LN1 sums: |sum|<=40, "
                            "err ~1e-4 of unit-scale x"):
                        nc.vector.tensor_reduce(
                            out=sums[:, sl], in_=xt, axis=AX.X, op=ALU.add)
                        sq = sqp.tile([P, XC, H], FP16, tag="sq")
                        nc.vector.tensor_tensor(sq, xt, xt, ALU.mult)
                        nc.vector.tensor_reduce(
                            out=ssq[:, sl], in_=sq, axis=AX.X, op=ALU.add)
                mu = stats.tile([P, GC], F32, tag="mu")
                nc.vector.tensor_scalar_mul(mu, sums, 1.0 / H)
                ex2 = stats.tile([P, GC], F32, tag="ex2")
                nc.vector.tensor_scalar(
                    out=ex2, in0=ssq, scalar1=1.0 / H, scalar2=EPS,
                    op0=ALU.mult, op1=ALU.add)
                var = stats.tile([P, GC], F32, tag="var")
                nc.vector.tensor_tensor(var, mu, mu, ALU.mult)
                nc.vector.tensor_tensor(var, ex2, var, ALU.subtract)
                std = stats.tile([P, GC], F32, tag="std")
                i_r = nc.scalar.activation(std, var, AF.Sqrt)
                act_phases[0].append(i_r)
                rst = stats.tile([P, GC], F32, tag="rstd")
                nc.vector.reciprocal_approx_fast(rst, std)
                mur = stats.tile([P, GC], F32, tag="mur")
                nc.vector.tensor_tensor(mur, mu, rst, ALU.mult)
                return rst, mur

            with (
                tc.tile_pool(name="ps1", bufs=BL, space="PSUM") as ps1,
                tc.tile_pool(name="pst", bufs=2, space="PSUM") as pst,
            ):
                psum1 = [ps1.tile([P, TM], F32, tag="ps1", name=f"ps1_{b}")
                         for b in range(BL)]
                for g in range(NG):
                    xn_g = {}
                    for b in range(BL):
                        rst, mur = emit_stats(b, g)
                        # normalize group g: xn = x*rstd - mu*rstd,
                        # two broadcast passes per x chunk
                        for q in range(SGRP):
                            qq = g * SGRP + q
                            sl = slice(q * XC, (q + 1) * XC)
                            xn = xnp.tile([P, XC, H], FP16, tag="xn")
                            rb = rst[:, sl][:, :, None].broadcast_to(
                                [P, XC, H])
                            mb = mur[:, sl][:, :, None].broadcast_to(
                                [P, XC, H])
                            nc.vector.tensor_tensor(
                                xn, x_sb[b][qq], rb, ALU.mult)
                            nc.vector.tensor_tensor(
                                xn, xn, mb, ALU.subtract)
                            xn_g[(b, q)] = xn
                    for c in range(g * GC, (g + 1) * GC):
                        q, r = divmod(c - g * GC, XC)
                        wq, wr = divmod(c, WC)
                        for b in range(BL):
                            nc.tensor.matmul(
                                psum1[b],
                                xn_g[(b, q)][:, r, :],
                                w1_sb[wq][:, wr, :],
                                start=(c == 0),
                                stop=(c == NC_TOK - 1),
                            )

                # ---- phase 2: token gelu, transpose, center ----------
                h1c = []  # per batch: list of KTM [P, P] fp16 tiles
                for b in range(BL):
                    h1 = small.tile([P, TM], F32, tag="h1")
                    if nontrivial_bias1:
                        nc.vector.tensor_scalar_mul(h1, psum1[b], g1_t)
                        nc.vector.tensor_add(h1, h1, bias1_sb)
                        i_g = nc.scalar.activation(h1, h1, AF.Gelu)
                    else:
                        i_g = nc.scalar.activation(h1, psum1[b], AF.Gelu,
                                                   scale=g1_t)
                    act_phases[1].append(i_g)

                    chunks = []
                    for k in range(KTM):
                        ps_t = pst.tile([P, P], F32, tag="pst")
                        nc.tensor.transpose(ps_t, h1[:, k * P:(k + 1) * P], ident)
                        h1T = small.tile([P, P], F32, tag="h1T")
                        nc.vector.tensor_copy(h1T, ps_t)
                        hsum = small.tile([P, 1], F32, tag="hsum")
                        nc.vector.tensor_reduce(
                            out=hsum, in_=h1T, axis=AX.X, op=ALU.add)
                        hmean = small.tile([P, 1], F32, tag="hmean")
                        nc.vector.tensor_scalar_mul(hmean, hsum, 1.0 / H)
                        hc = small.tile([P, P], FP16, tag="h1c")
                        nc.vector.tensor_scalar(
                            out=hc, in0=h1T, scalar1=hmean, scalar2=None,
                            op0=ALU.subtract)
                        chunks.append(hc)
                    h1c.append(chunks)

            ph1.close()  # release x/w1c/xn/stat SBUF for phase 3
            sq2p = ctx.enter_context(tc.tile_pool(name="sq2p", bufs=3))
            rstdp = ctx.enter_context(tc.tile_pool(name="rstdp", bufs=3))
            g2p = ctx.enter_context(tc.tile_pool(name="g2p", bufs=3))

            # ---- phase 3a: token matmul 2 + LN2 stats ----------------
            with (
                tc.tile_pool(name="ps2", bufs=4, space="PSUM") as ps2,
                tc.tile_pool(name="psv", bufs=2, space="PSUM") as psv,
            ):
                y2n = []
                for b in range(BL):
                    y2n.append(y2p.tile([P, N], FP16, tag="y2", name=f"y2n{b}"))

                p2s = {}
                vpair = None
                for j in range(NJ):
                    for b in range(BL):
                        idx = 2 * j + b
                        half = idx % 2
                        p2 = ps2.tile([P, 512], F32, tag="ps2")
                        p2s[idx] = p2
                        for k in range(KTM):
                            nc.tensor.matmul(
                                p2, h1c[b][k], w2_sb[j][:, k, :],
                                start=(k == 0), stop=(k == KTM - 1))
                        sq2 = sq2p.tile([P, 512], FP16, tag="sq2")
                        i_s = nc.scalar.activation(
                            sq2, p2, AF.Square, scale=float(1.0 / np.sqrt(H)))
                        act_phases[2].append(i_s)
                        if half == 0:
                            vpair = psv.tile([P, 1024], F32, tag="psv")
                        nc.tensor.matmul(
                            vpair[:, half * 512:(half + 1) * 512],
                            ones_sb, sq2, start=True, stop=True)
                        if half == 1:
                            std = rstdp.tile([P, 1024], F32, tag="std")
                            i_r = nc.scalar.activation(
                                std, vpair, AF.Sqrt, bias=eps_t)
                            act_phases[2].append(i_r)
                            rstd = rstdp.tile([P, 1024], F32, tag="rstd")
                            nc.vector.reciprocal_approx_fast(rstd, std)
                            for h2 in range(2):
                                i2 = idx - 1 + h2
                                j2, b2 = divmod(i2, 2)
                                nc.vector.tensor_tensor(
                                    y2n[b2][:, j2 * 512:(j2 + 1) * 512],
                                    p2s.pop(i2),
                                    rstd[:, h2 * 512:(h2 + 1) * 512],
                                    ALU.mult)

            # ---- phase 3b: channel MLP (half-hidden pipelining) ------
            with (
                tc.tile_pool(name="psr", bufs=3, space="PSUM") as psr,
                tc.tile_pool(name="pso", bufs=2, space="PSUM") as pso,
            ):
                for j in range(NJ):
                    for b in range(BL):
                        idx = 2 * j + b
                        y2s = y2n[b][:, j * 512:(j + 1) * 512]
                        g2 = g2p.tile([P, NCI, 512], FP16, tag="g2")
                        po = pso.tile([P, 512], F32, tag="pso")
                        for hh in range(2):
                            raw2 = psr.tile([P, 2, 512], F32, tag="psr")
                            for ci2 in range(2):
                                ci = hh * 2 + ci2
                                nc.tensor.matmul(
                                    raw2[:, ci2, :],
                                    cw1_sb[:, ci * P:(ci + 1) * P],
                                    y2s, start=True, stop=True)
                            if nontrivial_bias1:
                                for ci2 in range(2):
                                    ci = hh * 2 + ci2
                                    i_g = nc.scalar.activation(
                                        g2[:, ci, :],
                                        raw2[:, ci2, :],
                                        AF.Gelu, bias=cb1_sb[:, ci:ci + 1])
                                    act_phases[3].append(i_g)
                            else:
                                i_g = nc.scalar.activation(
                                    g2[:, hh * 2:(hh + 1) * 2, :], raw2,
                                    AF.Gelu)
                                act_phases[3].append(i_g)
                            for ci2 in range(2):
                                ci = hh * 2 + ci2
                                nc.tensor.matmul(
                                    po,
                                    cw2_sb[:, ci, :],
                                    g2[:, ci, :],
                                    start=(ci == 0), stop=(ci == NCI - 1))
                        osb = outp.tile([P, 512], FP16, tag="osb")
                        if idx % 2 == 0:
                            if nontrivial_cb2:
                                nc.vector.tensor_scalar(
                                    out=osb, in0=po, scalar1=cb2_t,
                                    scalar2=None, op0=ALU.add)
                            else:
                                nc.vector.tensor_copy(osb, po)
                        else:
                            i_c = nc.scalar.activation(
                                osb, po, AF.Copy, bias=0.0)
                            act_phases[3].append(i_c)
                            if nontrivial_cb2:
                                nc.vector.tensor_scalar(
                                    out=osb, in0=osb, scalar1=cb2_t,
                                    scalar2=None, op0=ALU.add)
                        nc.sync.dma_start(
                            out_v[b][:, j * 512:(j + 1) * 512], osb)

            # ---- ACT table-set ordering edges ------------------------
            for ph in range(3):
                for f in act_phases[ph + 1]:
                    for t in act_phases[ph]:
                        bass_rust.add_dep_helper(
                            f.ins, t.ins, sync=False,
                            reason="act table set phase ordering")

    nc.compile()
    return nc


def _host_prep(inputs):
    FP = np.float16

    x = np.asarray(inputs["x"], dtype=np.float32)
    ln1_g = np.asarray(inputs["ln1_g"], np.float32)
    ln1_b = np.asarray(inputs["ln1_b"], np.float32)
    ln2_g = np.asarray(inputs["ln2_g"], np.float32)
    ln2_b = np.asarray(inputs["ln2_b"], np.float32)
    tok_w1 = np.asarray(inputs["tok_w1"], np.float32)
    tok_b1 = np.asarray(inputs["tok_b1"], np.float32)
    tok_w2 = np.asarray(inputs["tok_w2"], np.float32)
    ch_w1 = np.asarray(inputs["ch_w1"], np.float32)
    ch_b1 = np.asarray(inputs["ch_b1"], np.float32)
    ch_w2 = np.asarray(inputs["ch_w2"], np.float32)
    ch_b2 = np.asarray(inputs["ch_b2"], np.float32)

    w1c = np.cumsum(tok_w1, axis=0, dtype=np.float64).astype(np.float32)
    colsum1 = w1c.sum(axis=0, dtype=np.float64).astype(np.float32)
    bias1 = ln1_b[:, None] * colsum1[None, :] + tok_b1[None, :]
    cw1 = (ln2_g[:, None] * ch_w1).astype(np.float32)
    cb1 = (ch_b1 + ch_w1.T @ ln2_b).astype(np.float32)

    nontrivial_bias1 = bool(np.any(bias1 != 0.0) or np.any(cb1 != 0.0))
    nontrivial_cb2 = bool(np.any(ch_b2 != 0.0))

    x_dev = np.ascontiguousarray(
        x.reshape(B, NC_TOK, P, H).transpose(0, 2, 1, 3)).astype(FP)
    w1c_dev = np.ascontiguousarray(
        w1c.reshape(NC_TOK, P, TM).transpose(1, 0, 2)).astype(FP)
    w2_dev = np.ascontiguousarray(
        tok_w2.reshape(KTM, P, NJ, 512).transpose(1, 0, 2, 3)).astype(FP)
    cw2_dev = np.ascontiguousarray(
        ch_w2.reshape(NCI, P, H).transpose(1, 0, 2)).astype(FP)

    shared = {
        "w1c": w1c_dev,
        "w2": w2_dev,
        "g1": ln1_g.reshape(P, 1).copy(),
        "bias1": np.ascontiguousarray(bias1, np.float32),
        "cw1": cw1.astype(FP),
        "cb1": np.ascontiguousarray(cb1.reshape(NCI, P).T.copy()),
        "cw2": cw2_dev,
        "cb2": ch_b2.reshape(P, 1).astype(np.float32).copy(),
        "ones": np.ones((P, P), FP),
    }
    return x_dev, shared, nontrivial_bias1, nontrivial_cb2


def _assemble(r):
    return r["out"].astype(np.float32).transpose(0, 2, 1)


def kernel(**inputs) -> np.ndarray:
    from concourse.bass_utils import run_bass_kernel_spmd

    x, shared, nb1, nb2 = _host_prep(inputs)

    key = (nb1, nb2)
    if key not in _cached:
        _cached[key] = _build(nb1, nb2)
    nc = _cached[key]

    in_maps = []
    for c in range(NCORES):
        m = dict(shared)
        m["x"] = np.ascontiguousarray(x[c * BL:(c + 1) * BL])
        in_maps.append(m)

    res = run_bass_kernel_spmd(nc, in_maps, core_ids=list(range(NCORES)))
    out = np.concatenate([_assemble(r) for r in res.results], axis=0)
    return np.ascontiguousarray(out, dtype=np.float32)


if __name__ == "__main__":
    rng = np.random.default_rng(0)
    ins = {
        "x": rng.standard_normal((B, N, H)).astype(np.float32),
        "ln1_g": np.ones(H, np.float32),
        "ln1_b": np.zeros(H, np.float32),
        "ln2_g": np.ones(H, np.float32),
        "ln2_b": np.zeros(H, np.float32),
        "tok_w1": (rng.standard_normal((N, TM)) * 0.02).astype(np.float32),
        "tok_b1": np.zeros(TM, np.float32),
        "tok_w2": (rng.standard_normal((TM, N)) * 0.02).astype(np.float32),
        "tok_b2": np.zeros(N, np.float32),
        "ch_w1": (rng.standard_normal((H, CM)) * 0.02).astype(np.float32),
        "ch_b1": np.zeros(CM, np.float32),
        "ch_w2": (rng.standard_normal((CM, H)) * 0.02).astype(np.float32),
        "ch_b2": np.zeros(H, np.float32),
    }
    out = kernel(**ins)
    print("out", out.shape, out.dtype)


# revision 20
# speedup vs baseline: 1.6814x; 1.1058x over previous
"""Trainium2 Bass kernel for nn_AutoregressiveMixerBlock.

Reference computation (per batch b):
  y  = LN_H(x)                                    # layer norm over H
  t  = revcumsum_N(y)                             # t[j] = sum_{i>=j} y[i]
  h  = gelu(t^T @ tok_w1 + tok_b1)                # [H, TM]
  y2 = (h @ tok_w2 + tok_b2)^T                    # [N, H]
  y3 = LN_H(y2)
  out = gelu(y3 @ ch_w1 + ch_b1) @ ch_w2 + ch_b2  # [N, H]

Key algebraic folds (exact in real arithmetic, applied on host):
  * revcumsum+matmul:  sum_j t[j,h] w1[j,m] = sum_i y[i,h] W1c[i,m]
    with W1c = cumsum(tok_w1, axis=0) -> no on-device cumsum at all.
  * LN1 gain/bias move past the token matmul.
  * tok_b2 and the LN2 mean both vanish by centering h^T by its
    per-row (over H) mean before the second token matmul.
  * LN2 gain/bias fold into ch_w1 / ch_b1.

Perf structure:
  * fp16 matmul operands everywhere (bf16 speed, 8x the mantissa;
    fp8 DoubleRow was tried and its quantization noise alone exceeds
    the 2e-2 error budget).
  * x / w1c / w2 stream in as independent chunk tiles; x first.
  * LN1 stats emission is interleaved with the token matmul per
    stats group, so the PE starts as soon as the first chunks land.
  * the LN1 normalize runs as two broadcast tensor_tensor passes per
    x chunk (xn = x*rstd - mu*rstd) instead of 128 tiny ops.
  * channel-MLP hidden runs in two [P,1024] halves, triple-buffered.
  * LN2 sqrt/reciprocal are batched in [P,1024] pairs; the output
    staging copy alternates between DVE and ACT.
  * output is written fp16 and converted on host.

Sharding: data-parallel over B across 8 cores (2 batches per core),
weights replicated.
"""

import numpy as np

B, N, H = 16, 8192, 128
TM, CM = 256, 512
EPS = 1e-5
NCORES = 8
BL = B // NCORES          # batches per core
P = 128                   # partitions
NC_TOK = N // P           # 64 token chunks of 128
NJ = N // 512             # 16 column chunks of 512
KTM = TM // P             # 2 k-chunks for the second token matmul
NCI = CM // P             # 4 chunks of the channel hidden dim
XQ = 8                    # x DMA/stat chunks per batch (of NC_TOK)
WQ = 16                   # w1c chunk tiles
SGRP = 2                  # x chunks per stats-combine group

_cached = {}


def _build(nontrivial_bias1, nontrivial_cb2):
    import concourse.bass as bass
    import concourse.mybir as mybir
    import concourse.tile as tile
    from concourse import bacc
    from concourse.masks import make_identity
    import bass_rust

    F32 = mybir.dt.float32
    FP16 = mybir.dt.float16
    AF = mybir.ActivationFunctionType
    ALU = mybir.AluOpType
    AX = mybir.AxisListType

    nc = bacc.Bacc()

    # ---- DRAM tensors -------------------------------------------------
    x_d = nc.dram_tensor("x", [BL, P, NC_TOK, H], FP16, kind="ExternalInput")
    w1c_d = nc.dram_tensor("w1c", [P, NC_TOK, TM], FP16, kind="ExternalInput")
    w2_d = nc.dram_tensor("w2", [P, KTM, NJ, 512], FP16, kind="ExternalInput")
    g1_d = nc.dram_tensor("g1", [P, 1], F32, kind="ExternalInput")
    bias1_d = nc.dram_tensor("bias1", [P, TM], F32, kind="ExternalInput")
    cw1_d = nc.dram_tensor("cw1", [H, CM], FP16, kind="ExternalInput")
    cb1_d = nc.dram_tensor("cb1", [P, NCI], F32, kind="ExternalInput")
    cw2_d = nc.dram_tensor("cw2", [P, NCI, H], FP16, kind="ExternalInput")
    cb2_d = nc.dram_tensor("cb2", [P, 1], F32, kind="ExternalInput")
    ones_d = nc.dram_tensor("ones", [P, P], FP16, kind="ExternalInput")
    out_d = nc.dram_tensor("out", [BL, H, N], FP16, kind="ExternalOutput")

    out_v = [out_d[b] for b in range(BL)]
    XC = NC_TOK // XQ     # token chunks per x chunk (8)
    WC = NC_TOK // WQ     # token chunks per w1c chunk (4)
    GC = XC * SGRP        # token chunks per stats group (16)
    NG = XQ // SGRP       # stats groups per batch (4)

    act_phases = [[], [], [], []]  # ACT table-set phase buckets

    with tile.TileContext(nc) as tc:
        import contextlib
        with contextlib.ExitStack() as ctx:
            const = ctx.enter_context(tc.tile_pool(name="const", bufs=1))
            y2p = ctx.enter_context(tc.tile_pool(name="y2p", bufs=BL))
            w2p = ctx.enter_context(tc.tile_pool(name="w2p", bufs=NJ))
            small = ctx.enter_context(tc.tile_pool(name="small", bufs=6))
            outp = ctx.enter_context(tc.tile_pool(name="outp", bufs=3))
            # phase-1-scoped pools: their SBUF is recycled for phase 3
            ph1 = contextlib.ExitStack()
            xall = ph1.enter_context(tc.tile_pool(name="xall", bufs=BL * XQ))
            w1p = ph1.enter_context(tc.tile_pool(name="w1p", bufs=WQ))
            stats = ph1.enter_context(
                tc.tile_pool(name="stats", bufs=2 * BL * NG))
            sqp = ph1.enter_context(tc.tile_pool(name="sqp", bufs=4))
            xnp = ph1.enter_context(tc.tile_pool(name="xnp", bufs=8))

            # ---- x DMAs first (LN1 is the startup critical path) -----
            x_sb = []     # [b][q] -> [P, XC, H] tile
            for b in range(BL):
                x_sb.append([])
                for q in range(XQ):
                    xt = xall.tile([P, XC, H], FP16, tag="xall",
                                   name=f"x{b}_{q}")
                    x_sb[b].append(xt)
            w1_sb = [None] * WQ
            for q in range(XQ):
                for b in range(BL):
                    if q < SGRP:
                        # first stats group: halve DMA latency by
                        # splitting across two queues
                        hx = XC // 2
                        for s in range(2):
                            nc.sync.dma_start(
                                x_sb[b][q][:, s * hx:(s + 1) * hx, :],
                                x_d[b, :, q * XC + s * hx:
                                    q * XC + (s + 1) * hx, :])
                    else:
                        nc.sync.dma_start(
                            x_sb[b][q], x_d[b, :, q * XC:(q + 1) * XC, :])
                for i in (2 * q, 2 * q + 1):
                    wt = w1p.tile([P, WC, TM], FP16, tag="w1", name=f"w1_{i}")
                    nc.sync.dma_start(wt, w1c_d[:, i * WC:(i + 1) * WC, :])
                    w1_sb[i] = wt
            w2_sb = []
            for j in range(NJ):
                wt = w2p.tile([P, KTM, 512], FP16, tag="w2", name=f"w2_{j}")
                nc.sync.dma_start(wt, w2_d[:, :, j, :])
                w2_sb.append(wt)

            # ---- constants -------------------------------------------
            g1_sb = const.tile([P, 1], F32)
            nc.sync.dma_start(g1_sb, g1_d[:])
            cw1_sb = const.tile([H, CM], FP16)
            nc.sync.dma_start(cw1_sb, cw1_d[:])
            cb1_sb = const.tile([P, NCI], F32)
            nc.sync.dma_start(cb1_sb, cb1_d[:])
            cw2_sb = const.tile([P, NCI, H], FP16)
            nc.sync.dma_start(cw2_sb, cw2_d[:])
            ones_sb = const.tile([P, P], FP16)
            nc.sync.dma_start(ones_sb, ones_d[:])
            ident = const.tile([P, P], F32)
            make_identity(nc, ident)
            if nontrivial_bias1:
                bias1_sb = const.tile([P, TM], F32)
                nc.sync.dma_start(bias1_sb, bias1_d[:])
            if nontrivial_cb2:
                cb2_sb = const.tile([P, 1], F32)
                nc.sync.dma_start(cb2_sb, cb2_d[:])
                cb2_t = small.tile([P, 1], F32, tag="cb2t")
                nc.vector.tensor_copy(cb2_t, cb2_sb)
            g1_t = small.tile([P, 1], F32)
            nc.vector.tensor_copy(g1_t, g1_sb)
            eps_t = const.tile([P, 1], F32)
            nc.vector.memset(eps_t, EPS)

            # ---- phase 1: LN1 + token matmul 1, group-interleaved ----
            def emit_stats(b, g):
                """LN1 stats for chunk group g of batch b ->
                (rstd [P,GC], nmur=-mu*rstd [P,GC])."""
                sums = stats.tile([P, GC], F32, tag="sums")
                ssq = stats.tile([P, GC], F32, tag="ssq")
                for q in range(SGRP):
                    xt = x_sb[b][g * SGRP + q]
                    sl = slice(q * XC, (q + 1) * XC)
                    nc.vector.tensor_reduce(
                        out=sums[:, sl], in_=xt, axis=AX.X, op=ALU.add)
                    sq = sqp.tile([P, XC, H], FP16, tag="sq")
                    nc.vector.tensor_tensor(sq, xt, xt, ALU.mult)
                    nc.vector.tensor_reduce(
                        out=ssq[:, sl], in_=sq, axis=AX.X, op=ALU.add)
                nmu = stats.tile([P, GC], F32, tag="nmu")
                nc.vector.tensor_scalar_mul(nmu, sums, -1.0 / H)
                ex2 = stats.tile([P, GC], F32, tag="ex2")
                nc.vector.tensor_scalar(
                    out=ex2, in0=ssq, scalar1=1.0 / H, scalar2=EPS,
                    op0=ALU.mult, op1=ALU.add)
                var = stats.tile([P, GC], F32, tag="var")
                nc.vector.tensor_tensor(var, nmu, nmu, ALU.mult)
                nc.vector.tensor_tensor(var, ex2, var, ALU.subtract)
                std = stats.tile([P, GC], F32, tag="std")
                i_r = nc.scalar.activation(std, var, AF.Sqrt)
                act_phases[0].append(i_r)
                rst = stats.tile([P, GC], F32, tag="rstd")
                nc.vector.reciprocal_approx_fast(rst, std)
                nmur = stats.tile([P, GC], F32, tag="nmur")
                nc.vector.tensor_tensor(nmur, nmu, rst, ALU.mult)
                return rst, nmur

            with (
                tc.tile_pool(name="ps1", bufs=BL, space="PSUM") as ps1,
                tc.tile_pool(name="pst", bufs=2, space="PSUM") as pst,
            ):
                psum1 = [ps1.tile([P, TM], F32, tag="ps1", name=f"ps1_{b}")
                         for b in range(BL)]
                for g in range(NG):
                    st_g = {}
                    for b in range(BL):
                        st_g[b] = emit_stats(b, g)
                    for c in range(g * GC, (g + 1) * GC):
                        gr = c - g * GC
                        q, r = divmod(gr, XC)
                        wq, wr = divmod(c, WC)
                        for b in range(BL):
                            rst, nmur = st_g[b]
                            # xn = x*rstd - mu*rstd on ACT (Identity is
                            # in every activation table: no table cost)
                            xn = xnp.tile([P, P], FP16, tag="xn")
                            nc.scalar.activation(
                                xn, x_sb[b][g * SGRP + q][:, r, :],
                                AF.Identity,
                                scale=rst[:, gr:gr + 1],
                                bias=nmur[:, gr:gr + 1])
                            nc.tensor.matmul(
                                psum1[b],
                                xn,
                                w1_sb[wq][:, wr, :],
                                start=(c == 0),
                                stop=(c == NC_TOK - 1),
                            )

                # ---- phase 2: token gelu, transpose, center ----------
                h1c = []  # per batch: list of KTM [P, P] fp16 tiles
                for b in range(BL):
                    h1 = small.tile([P, TM], F32, tag="h1")
                    if nontrivial_bias1:
                        nc.vector.tensor_scalar_mul(h1, psum1[b], g1_t)
                        nc.vector.tensor_add(h1, h1, bias1_sb)
                        i_g = nc.scalar.activation(h1, h1, AF.Gelu)
                    else:
                        i_g = nc.scalar.activation(h1, psum1[b], AF.Gelu,
                                                   scale=g1_t)
                    act_phases[1].append(i_g)

                    chunks = []
                    for k in range(KTM):
                        ps_t = pst.tile([P, P], F32, tag="pst")
                        nc.tensor.transpose(ps_t, h1[:, k * P:(k + 1) * P], ident)
                        h1T = small.tile([P, P], F32, tag="h1T")
                        nc.vector.tensor_copy(h1T, ps_t)
                        hsum = small.tile([P, 1], F32, tag="hsum")
                        nc.vector.tensor_reduce(
                            out=hsum, in_=h1T, axis=AX.X, op=ALU.add)
                        hmean = small.tile([P, 1], F32, tag="hmean")
                        nc.vector.tensor_scalar_mul(hmean, hsum, 1.0 / H)
                        hc = small.tile([P, P], FP16, tag="h1c")
                        nc.vector.tensor_scalar(
                            out=hc, in0=h1T, scalar1=hmean, scalar2=None,
                            op0=ALU.subtract)
                        chunks.append(hc)
                    h1c.append(chunks)

            ph1.close()  # release x/w1c/xn/stat SBUF for phase 3
            sq2p = ctx.enter_context(tc.tile_pool(name="sq2p", bufs=3))
            rstdp = ctx.enter_context(tc.tile_pool(name="rstdp", bufs=3))
            g2p = ctx.enter_context(tc.tile_pool(name="g2p", bufs=3))

            # ---- phase 3a: token matmul 2 + LN2 stats ----------------
            with (
                tc.tile_pool(name="ps2", bufs=4, space="PSUM") as ps2,
                tc.tile_pool(name="psv", bufs=2, space="PSUM") as psv,
            ):
                y2n = []
                for b in range(BL):
                    y2n.append(y2p.tile([P, N], FP16, tag="y2", name=f"y2n{b}"))

                p2s = {}
                vpair = None
                for j in range(NJ):
                    for b in range(BL):
                        idx = 2 * j + b
                        half = idx % 2
                        p2 = ps2.tile([P, 512], F32, tag="ps2")
                        p2s[idx] = p2
                        for k in range(KTM):
                            nc.tensor.matmul(
                                p2, h1c[b][k], w2_sb[j][:, k, :],
                                start=(k == 0), stop=(k == KTM - 1))
                        sq2 = sq2p.tile([P, 512], FP16, tag="sq2")
                        i_s = nc.scalar.activation(
                            sq2, p2, AF.Square, scale=float(1.0 / np.sqrt(H)))
                        act_phases[2].append(i_s)
                        if half == 0:
                            vpair = psv.tile([P, 1024], F32, tag="psv")
                        nc.tensor.matmul(
                            vpair[:, half * 512:(half + 1) * 512],
                            ones_sb, sq2, start=True, stop=True)
                        if half == 1:
                            std = rstdp.tile([P, 1024], F32, tag="std")
                            i_r = nc.scalar.activation(
                                std, vpair, AF.Sqrt, bias=eps_t)
                            act_phases[2].append(i_r)
                            rstd = rstdp.tile([P, 1024], F32, tag="rstd")
                            nc.vector.reciprocal_approx_fast(rstd, std)
                            for h2 in range(2):
                                i2 = idx - 1 + h2
                                j2, b2 = divmod(i2, 2)
                                nc.vector.tensor_tensor(
                                    y2n[b2][:, j2 * 512:(j2 + 1) * 512],
                                    p2s.pop(i2),
                                    rstd[:, h2 * 512:(h2 + 1) * 512],
                                    ALU.mult)

            # ---- phase 3b: channel MLP (half-hidden pipelining) ------
            with (
                tc.tile_pool(name="psr", bufs=3, space="PSUM") as psr,
                tc.tile_pool(name="pso", bufs=2, space="PSUM") as pso,
            ):
                for j in range(NJ):
                    for b in range(BL):
                        idx = 2 * j + b
                        y2s = y2n[b][:, j * 512:(j + 1) * 512]
                        g2 = g2p.tile([P, NCI, 512], FP16, tag="g2")
                        po = pso.tile([P, 512], F32, tag="pso")
                        for hh in range(2):
                            raw2 = psr.tile([P, 2, 512], F32, tag="psr")
                            for ci2 in range(2):
                                ci = hh * 2 + ci2
                                nc.tensor.matmul(
                                    raw2[:, ci2, :],
                                    cw1_sb[:, ci * P:(ci + 1) * P],
                                    y2s, start=True, stop=True)
                            if nontrivial_bias1:
                                for ci2 in range(2):
                                    ci = hh * 2 + ci2
                                    i_g = nc.scalar.activation(
                                        g2[:, ci, :],
                                        raw2[:, ci2, :],
                                        AF.Gelu, bias=cb1_sb[:, ci:ci + 1])
                                    act_phases[3].append(i_g)
                            else:
                                i_g = nc.scalar.activation(
                                    g2[:, hh * 2:(hh + 1) * 2, :], raw2,
                                    AF.Gelu)
                                act_phases[3].append(i_g)
                            for ci2 in range(2):
                                ci = hh * 2 + ci2
                                nc.tensor.matmul(
                                    po,
                                    cw2_sb[:, ci, :],
                                    g2[:, ci, :],
                                    start=(ci == 0), stop=(ci == NCI - 1))
                        osb = outp.tile([P, 512], FP16, tag="osb")
                        if nontrivial_cb2:
                            nc.vector.tensor_scalar(
                                out=osb, in0=po, scalar1=cb2_t,
                                scalar2=None, op0=ALU.add)
                        else:
                            nc.vector.tensor_copy(osb, po)
                        nc.sync.dma_start(
                            out_v[b][:, j * 512:(j + 1) * 512], osb)

            # ---- ACT table-set ordering edges ------------------------
            for ph in range(3):
                for f in act_phases[ph + 1]:
                    for t in act_phases[ph]:
                        bass_rust.add_dep_helper(
                            f.ins, t.ins, sync=False,
                            reason="act table set phase ordering")

    nc.compile()
    return nc


def _host_prep(inputs):
    FP = np.float16

    x = np.asarray(inputs["x"], dtype=np.float32)
    ln1_g = np.asarray(inputs["ln1_g"], np.float32)
    ln1_b = np.asarray(inputs["ln1_b"], np.float32)
    ln2_g = np.asarray(inputs["ln2_g"], np.float32)
    ln2_b = np.asarray(inputs["ln2_b"], np.float32)
    tok_w1 = np.asarray(inputs["tok_w1"], np.float32)
    tok_b1 = np.asarray(inputs["tok_b1"], np.float32)
    tok_w2 = np.asarray(inputs["tok_w2"], np.float32)
    ch_w1 = np.asarray(inputs["ch_w1"], np.float32)
    ch_b1 = np.asarray(inputs["ch_b1"], np.float32)
    ch_w2 = np.asarray(inputs["ch_w2"], np.float32)
    ch_b2 = np.asarray(inputs["ch_b2"], np.float32)

    w1c = np.cumsum(tok_w1, axis=0, dtype=np.float64).astype(np.float32)
    colsum1 = w1c.sum(axis=0, dtype=np.float64).astype(np.float32)
    bias1 = ln1_b[:, None] * colsum1[None, :] + tok_b1[None, :]
    cw1 = (ln2_g[:, None] * ch_w1).astype(np.float32)
    cb1 = (ch_b1 + ch_w1.T @ ln2_b).astype(np.float32)

    nontrivial_bias1 = bool(np.any(bias1 != 0.0) or np.any(cb1 != 0.0))
    nontrivial_cb2 = bool(np.any(ch_b2 != 0.0))

    x_dev = np.ascontiguousarray(
        x.reshape(B, NC_TOK, P, H).transpose(0, 2, 1, 3)).astype(FP)
    w1c_dev = np.ascontiguousarray(
        w1c.reshape(NC_TOK, P, TM).transpose(1, 0, 2)).astype(FP)
    w2_dev = np.ascontiguousarray(
        tok_w2.reshape(KTM, P, NJ, 512).transpose(1, 0, 2, 3)).astype(FP)
    cw2_dev = np.ascontiguousarray(
        ch_w2.reshape(NCI, P, H).transpose(1, 0, 2)).astype(FP)

    shared = {
        "w1c": w1c_dev,
        "w2": w2_dev,
        "g1": ln1_g.reshape(P, 1).copy(),
        "bias1": np.ascontiguousarray(bias1, np.float32),
        "cw1": cw1.astype(FP),
        "cb1": np.ascontiguousarray(cb1.reshape(NCI, P).T.copy()),
        "cw2": cw2_dev,
        "cb2": ch_b2.reshape(P, 1).astype(np.float32).copy(),
        "ones": np.ones((P, P), FP),
    }
    return x_dev, shared, nontrivial_bias1, nontrivial_cb2


def _assemble(r):
    return r["out"].astype(np.float32).transpose(0, 2, 1)


def kernel(**inputs) -> np.ndarray:
    from concourse.bass_utils import run_bass_kernel_spmd

    x, shared, nb1, nb2 = _host_prep(inputs)

    key = (nb1, nb2)
    if key not in _cached:
        _cached[key] = _build(nb1, nb2)
    nc = _cached[key]

    in_maps = []
    for c in range(NCORES):
        m = dict(shared)
        m["x"] = np.ascontiguousarray(x[c * BL:(c + 1) * BL])
        in_maps.append(m)

    res = run_bass_kernel_spmd(nc, in_maps, core_ids=list(range(NCORES)))
    out = np.concatenate([_assemble(r) for r in res.results], axis=0)
    return np.ascontiguousarray(out, dtype=np.float32)


if __name__ == "__main__":
    rng = np.random.default_rng(0)
    ins = {
        "x": rng.standard_normal((B, N, H)).astype(np.float32),
        "ln1_g": np.ones(H, np.float32),
        "ln1_b": np.zeros(H, np.float32),
        "ln2_g": np.ones(H, np.float32),
        "ln2_b": np.zeros(H, np.float32),
        "tok_w1": (rng.standard_normal((N, TM)) * 0.02).astype(np.float32),
        "tok_b1": np.zeros(TM, np.float32),
        "tok_w2": (rng.standard_normal((TM, N)) * 0.02).astype(np.float32),
        "tok_b2": np.zeros(N, np.float32),
        "ch_w1": (rng.standard_normal((H, CM)) * 0.02).astype(np.float32),
        "ch_b1": np.zeros(CM, np.float32),
        "ch_w2": (rng.standard_normal((CM, H)) * 0.02).astype(np.float32),
        "ch_b2": np.zeros(H, np.float32),
    }
    out = kernel(**ins)
    print("out", out.shape, out.dtype)
